# revision 22
# baseline (speedup 1.0000x reference)
"""Trainium2 Bass kernel for nn_Attribution (sparse local-window attention).

Data-parallel over batch n=8 -> one batch element per NeuronCore.

Per-core computation (c_in=256, ch=128, 64x64 image):
    h    = W1 @ x + b1
    corr = 5x5 local window correlation of h (zero padded), /sqrt(128)
    attn = softmax over the 25 window entries
    samp = sum_k attn_k * shift_k(h)
    gate = sigmoid(relu(W2 @ h + b2)) = 0.5 + 0.5*relu(tanh((z+b2)/2))
    out  = Wout @ (gate * samp) + bout

Layout: positions flattened row-major with 2 zero-pad rows top/bottom
(68 rows x 64 = 4352 positions = 34 chunks of 128).  Scores "born
transposed" (keys of chunk c on partitions, queries on free axis).
Out-of-window entries killed by a {0,1} mask after exp; out-of-image x
neighbors accounted by denominator correction D (exp(0)=1 each in the
zero-padded reference).

This version keeps the PE stream minimal and the output phase fused into
the chunk pipeline:
  - den/samp PSUM banks are pre-zeroed by memsets on DVE/GPSIMD (idle
    engines), so every den/samp matmul is a plain accumulate and the 16
    pre-zero PE matmuls of the previous version are gone,
  - den uses an all-ones [128,128] stationary so the column sums land
    broadcast across all partitions: the reciprocal is computed full-width
    on DVE and the PE partition-broadcast matmuls are gone,
  - each group's normalize + output conv + bias + store is emitted as soon
    as its denominator closes, so output DMA streams during the chunk
    pipeline instead of draining at the end,
  - evacuations are spread: ACT does only exp/tanh, DVE does mask/recip/
    normalize/conv1-bias, GPSIMD does transpose-evac/gate/attr/out-bias,
  - input rides in 8 fat x DMAs + 2 weight + 2 blob DMAs over 4 queues.
"""
import sys

sys.path.insert(0, "/opt/trn_rl_repo")

import numpy as np
import ml_dtypes

import concourse.bass as bass
import concourse.mybir as mybir
import concourse.tile as tile
from concourse import bacc
from concourse.bass_utils import run_bass_kernel_spmd

F32 = mybir.dt.float32
BF16 = mybir.dt.bfloat16
AF = mybir.ActivationFunctionType
ALU = mybir.AluOpType

N, CIN, CH, H, W = 8, 256, 128, 64, 64
HW = H * W                      # 4096
RAD = 2
KROWS = H + 2 * RAD             # 68 padded rows
PADPOS = KROWS * W              # 4352
NCHUNK = PADPOS // 128          # 34 key chunks (2 rows each)
NSUB = H // 2                   # 32 query subs (128 queries each)
NGRP = NSUB // 4                # 8 groups of 4 subs (one PSUM bank each)
SCALE = 1.0 / np.sqrt(np.float32(CH))

# ---- const blob layout (bf16 [128, BLOBW]) ----
O_W2T = 0            # [128,128]
O_WOT = 128          # [128,256]
O_M2G = 384          # maskC2g [128,896]
O_IDENT = 1280       # [128,128]
O_ONESB = 1408       # [128,128] all-ones
O_B1 = 1536          # [128,1] f32 (2 bf16 cols)
O_B2H = 1538         # [128,1] f32
O_BOUT = 1540        # [128,2] f32 (4 bf16 cols)
O_D2H = 1544         # [128,512] bf16: D(q) tiled, identical rows
BLOBW = 2056


def _build_masks():
    """maskC2g: (128, 896) {0,1} bf16 = maskC | zeros(128) | maskC.
    maskC col 128*a+q is key (chunk c, pos p) vs query q of sub s=c-2+a:
    valid iff |2-2a + p//64 - q//64| <= 2 and |p%64 - q%64| <= 2."""
    m = np.zeros((128, 384), dtype=np.float32)
    for a in range(3):
        for p in range(128):
            for q in range(128):
                dy = 2 - 2 * a + p // 64 - q // 64
                if abs(dy) <= RAD and abs(p % 64 - q % 64) <= RAD:
                    m[p, 128 * a + q] = 1.0
    m2g = np.concatenate([m, np.zeros((128, 128), np.float32), m], axis=1)

    cnt = np.array([sum(1 for dx in range(-RAD, RAD + 1) if not 0 <= qx + dx < W)
                    for qx in range(W)], dtype=np.float32)
    drow = 5.0 * np.concatenate([cnt, cnt])                 # (128,) D
    d2h = np.tile(np.tile(drow, 4)[None, :], (128, 1))      # (128,512)
    return m2g.astype(ml_dtypes.bfloat16), d2h.astype(ml_dtypes.bfloat16)


def _chunk_parts(c):
    """den/samp MM parts for chunk c: (g, s_lo, s_hi) sub-ranges split at
    4-sub PSUM bank boundaries.  Banks are pre-zeroed by memsets, so every
    part is a plain accumulate."""
    smin, smax = max(0, c - 2), min(NSUB - 1, c)
    parts = []
    for g in range(smin // 4, smax // 4 + 1):
        parts.append((g, max(smin, 4 * g), min(smax, 4 * g + 3)))
    return parts


def build_nc(repeat=1, sim_safe=False):
    nc = bacc.Bacc("TRN2", target_bir_lowering=False, debug=False, num_devices=8)

    x_d = nc.declare_dram_parameter("x", [CIN, HW], BF16, isOutput=False)
    w1t_d = nc.declare_dram_parameter("W1T", [CIN, CH], BF16, isOutput=False)
    blob_d = nc.declare_dram_parameter("blob", [128, BLOBW], BF16, isOutput=False)
    out_d = nc.declare_dram_parameter("out", [CIN, HW], BF16, isOutput=True)

    with tile.TileContext(nc) as tc:
        with (
            tc.tile_pool(name="per", bufs=1) as per,
            tc.tile_pool(name="smp", bufs=8) as smp,
            tc.tile_pool(name="otp", bufs=4) as otp,
            tc.tile_pool(name="pA", bufs=2, space="PSUM") as pA,   # 2x[128,1024] f32
            tc.tile_pool(name="pB", bufs=2, space="PSUM") as pB,   # 2x[128,512] f32 samp
            tc.tile_pool(name="pD", bufs=2, space="PSUM") as pD,   # 2x[128,512] f32 den
        ):
            blobw = per.tile([128, 2 * CH], BF16, tag="blobw")
            blob = per.tile([128, BLOBW], BF16, tag="blob")
            xall = per.tile([128, 2 * HW], BF16, tag="xall")
            hpad = per.tile([128, PADPOS], BF16, tag="hpad")
            hT = per.tile([128, PADPOS], BF16, tag="hT")
            attnm = per.tile([128, NCHUNK * 512], BF16, tag="attnm")
            Pg = per.tile([128, HW], BF16, tag="Pg")
            attr = per.tile([128, HW], BF16, tag="attr")

            w1t0 = blobw[:, 0:CH]
            w1t1 = blobw[:, CH:2 * CH]
            w2t = blob[:, O_W2T:O_W2T + 128]
            wot = blob[:, O_WOT:O_WOT + 256]
            maskC2g = blob[:, O_M2G:O_M2G + 896]
            maskC = blob[:, O_M2G:O_M2G + 384]
            ident = blob[:, O_IDENT:O_IDENT + 128]
            onesb = blob[:, O_ONESB:O_ONESB + 128]
            b1 = blob[:, O_B1:O_B1 + 2].bitcast(F32)
            b2h = blob[:, O_B2H:O_B2H + 2].bitcast(F32)
            bout0 = blob[:, O_BOUT:O_BOUT + 2].bitcast(F32)
            bout1 = blob[:, O_BOUT + 2:O_BOUT + 4].bitcast(F32)
            d2h = blob[:, O_D2H:O_D2H + 512]

            # --- input DMAs over 4 issue queues.  Per queue: the weight /
            # blob piece that queue owns, then x blocks in consumption
            # order.  Each dma_start is packetized across all 16 HW DMA
            # engines, so few fat transfers saturate the ~250GB/s link.
            def xdma(eng, half, u):
                src = x_d[128 * half:128 * (half + 1), 1024 * u:1024 * (u + 1)]
                eng.dma_start(
                    xall[:, HW * half + 1024 * u:HW * half + 1024 * (u + 1)], src)

            nc.sync.dma_start(blobw[:, 0:CH], w1t_d[0:128, :])
            nc.scalar.dma_start(blobw[:, CH:2 * CH], w1t_d[128:256, :])
            # first half-block finer so the first conv matmul starts early
            nc.sync.dma_start(xall[:, 0:512], x_d[0:128, 0:512])
            nc.scalar.dma_start(xall[:, HW:HW + 512], x_d[128:256, 0:512])
            nc.gpsimd.dma_start(blob[:, O_IDENT:BLOBW], blob_d[:, O_IDENT:BLOBW])
            nc.sync.dma_start(xall[:, 512:1024], x_d[0:128, 512:1024])
            nc.scalar.dma_start(xall[:, HW + 512:HW + 1024], x_d[128:256, 512:1024])
            xdma(nc.gpsimd, 0, 1)
            xdma(nc.sync, 1, 1)
            nc.scalar.dma_start(blob[:, 0:O_IDENT], blob_d[:, 0:O_IDENT])
            xdma(nc.gpsimd, 0, 2)
            xdma(nc.sync, 1, 2)
            xdma(nc.scalar, 0, 3)
            xdma(nc.gpsimd, 1, 3)

            # pad chunks (0 and 33) are identically zero
            nc.vector.memset(hpad[:, 0:128], 0.0)
            nc.vector.memset(hpad[:, PADPOS - 128:PADPOS], 0.0)
            nc.gpsimd.memset(hT[:, 0:128], 0.0)
            nc.gpsimd.memset(hT[:, PADPOS - 128:PADPOS], 0.0)

            for _rep in range(repeat):
                # ---- P1: conv1 + transposes + conv2, PE kept streaming.
                def emit_transp_group(u):
                    pt = pA.tile([128, 1024], BF16, tag="pa", name=f"pt{u}")
                    for k in range(8):
                        c = 8 * u + 1 + k
                        nc.tensor.transpose(pt[:, 128 * k:128 * (k + 1)],
                                            hpad[:, 128 * c:128 * (c + 1)],
                                            ident)
                    nc.scalar.copy(hT[:, 128 * (8 * u + 1):128 * (8 * u + 9)],
                                   pt[:])

                def emit_conv2(b):
                    pz = pB.tile([128, 512], F32, tag="pb", name=f"pz{b}")
                    nc.tensor.matmul(pz[:], w2t,
                                     hpad[:, 128 + 512 * b:128 + 512 * (b + 1)],
                                     start=True, stop=True)
                    tg = smp.tile([128, 512], BF16, tag="tg")
                    nc.scalar.activation(tg[:], pz[:], AF.Tanh, scale=0.5, bias=b2h)
                    nc.vector.tensor_scalar(
                        out=Pg[:, 512 * b:512 * (b + 1)], in0=tg[:],
                        scalar1=0.0, scalar2=1.0, op0=ALU.max, op1=ALU.add)

                for u in range(4):
                    cvt = pA.tile([128, 1024], F32, tag="pa", name=f"cv{u}")
                    for h2 in range(2):
                        dst = cvt[:, 512 * h2:512 * (h2 + 1)]
                        cs = slice(1024 * u + 512 * h2, 1024 * u + 512 * (h2 + 1))
                        cs2 = slice(HW + cs.start, HW + cs.stop)
                        nc.tensor.matmul(dst, w1t0, xall[:, cs], start=True, stop=False)
                        nc.tensor.matmul(dst, w1t1, xall[:, cs2], start=False, stop=True)
                    nc.vector.tensor_scalar(
                        out=hpad[:, 128 + 1024 * u:128 + 1024 * (u + 1)],
                        in0=cvt[:], scalar1=b1, scalar2=None, op0=ALU.add)
                    if u >= 1:
                        emit_transp_group(u - 1)
                        emit_conv2(2 * (u - 1))
                        emit_conv2(2 * (u - 1) + 1)
                emit_transp_group(3)
                emit_conv2(6)
                emit_conv2(7)

                # ---- P2: chunk pipeline: scores/exp/mask with den+samp
                # matmuls trailing two pairs behind on the PE; each group's
                # recip/normalize/output conv/store fires as soon as its
                # denominator closes.
                deng = {}
                sampg = {}

                def ensure_group(g):
                    if g in deng or g >= NGRP:
                        return
                    deng[g] = pD.tile([128, 512], F32, tag="pd", name=f"dn{g}")
                    sampg[g] = pB.tile([128, 512], F32, tag="pb", name=f"sp{g}")
                    # den preset = D(q) (out-of-image correction), samp = 0;
                    # all den/samp matmuls then accumulate on top.
                    nc.scalar.copy(deng[g][:], d2h)
                    nc.vector.memset(sampg[g][:], 0.0)

                def emit_score_pair(cp):
                    sc = pA.tile([128, 1024], F32, tag="pa", name=f"sc{cp}")
                    spans = []
                    for ci in range(2):
                        c = 2 * cp + ci
                        lo, hi = max(0, c - 2), min(NSUB - 1, c)
                        alo = lo - (c - 2)
                        spans.append((alo, alo + hi - lo + 1))
                        nc.tensor.matmul(
                            sc[:, 512 * ci + 128 * alo:512 * ci + 128 * (alo + hi - lo + 1)],
                            hpad[:, 128 * c:128 * (c + 1)],
                            hpad[:, 128 * (lo + 1):128 * (hi + 2)],
                            start=True, stop=True)
                    # half the mask-mults ride on the otherwise idle gpsimd
                    # (slower per op, but with the 3-pair lag they have slack)
                    meng = nc.gpsimd if cp % 2 == 0 else nc.vector
                    if spans == [(0, 3), (0, 3)]:
                        asl = attnm[:, 1024 * cp:1024 * cp + 896]
                        nc.scalar.activation(asl, sc[:, 0:896], AF.Exp,
                                             scale=float(SCALE))
                        meng.tensor_tensor(out=asl, in0=asl,
                                           in1=maskC2g, op=ALU.mult)
                    else:
                        for ci, (a0, a1) in enumerate(spans):
                            ss = slice(512 * ci + 128 * a0, 512 * ci + 128 * a1)
                            asl = attnm[:, 1024 * cp + ss.start:1024 * cp + ss.stop]
                            nc.scalar.activation(asl, sc[:, ss], AF.Exp,
                                                 scale=float(SCALE))
                            nc.vector.tensor_tensor(
                                out=asl, in0=asl,
                                in1=maskC[:, 128 * a0:128 * a1], op=ALU.mult)

                def emit_densamp_chunk(c):
                    parts = _chunk_parts(c)
                    for g, s, e in parts:
                        aa = s - (c - 2)
                        rhs = attnm[:, 512 * c + 128 * aa:512 * c + 128 * (aa + e - s + 1)]
                        nc.tensor.matmul(
                            deng[g][:, 128 * (s - 4 * g):128 * (e + 1 - 4 * g)],
                            onesb, rhs, start=False, stop=False,
                            skip_group_check=True)
                    for g, s, e in parts:
                        aa = s - (c - 2)
                        nc.tensor.matmul(
                            sampg[g][:, 128 * (s - 4 * g):128 * (e + 1 - 4 * g)],
                            hT[:, 128 * c:128 * (c + 1)],
                            attnm[:, 512 * c + 128 * aa:512 * c + 128 * (aa + e - s + 1)],
                            start=False, stop=False, skip_group_check=True)

                def emit_finish_group(g):
                    gsl = slice(512 * g, 512 * (g + 1))
                    # z = 1 / (den + D): den banks were preset with D, so a
                    # plain full-width reciprocal does it (den is broadcast
                    # across partitions by the ones stationary).  The
                    # softmax 1/2 vs gate 2x cancels via Wout/2 on host.
                    z = smp.tile([128, 512], F32, tag="z", name=f"z{g}")
                    nc.vector.reciprocal_approx_fast(z[:], deng[g][:])
                    # attr = (samp * Pg) * z ; the z-normalize is all-SBUF so
                    # it can ride on gpsimd
                    nc.vector.tensor_tensor(out=attr[:, gsl], in0=sampg[g][:],
                                            in1=Pg[:, gsl], op=ALU.mult)
                    nc.vector.tensor_tensor(out=attr[:, gsl], in0=attr[:, gsl],
                                            in1=z[:], op=ALU.mult)
                    # output conv + bias + store for this group
                    po = pA.tile([128, 1024], F32, tag="pa", name=f"po{g}")
                    nc.tensor.matmul(po[:, 0:512], wot[:, 0:128], attr[:, gsl],
                                     start=True, stop=True)
                    nc.tensor.matmul(po[:, 512:1024], wot[:, 128:256], attr[:, gsl],
                                     start=True, stop=True)
                    ot = otp.tile([128, 1024], BF16, tag="ot")
                    if g % 2 == 0:
                        nc.vector.tensor_scalar(out=ot[:, 0:512], in0=po[:, 0:512],
                                                scalar1=bout0, scalar2=None,
                                                op0=ALU.add)
                        nc.scalar.activation(ot[:, 512:1024], po[:, 512:1024],
                                             AF.Identity, bias=bout1, scale=1.0)
                    else:
                        nc.scalar.activation(ot[:, 0:512], po[:, 0:512],
                                             AF.Identity, bias=bout0, scale=1.0)
                        nc.vector.tensor_scalar(out=ot[:, 512:1024],
                                                in0=po[:, 512:1024],
                                                scalar1=bout1, scalar2=None,
                                                op0=ALU.add)
                    nsp = 2 if g == NGRP - 1 else 1
                    for oc in range(2):
                        osl = slice(512 * oc, 512 * (oc + 1))
                        for j in range(nsp):
                            w = 512 // nsp
                            qcs = slice(512 * g + w * j, 512 * g + w * (j + 1))
                            ts = slice(osl.start + w * j, osl.start + w * (j + 1))
                            nc.sync.dma_start(out_d[128 * oc:128 * (oc + 1), qcs],
                                              ot[:, ts])

                ensure_group(0)
                ensure_group(1)
                for cp in range(20):
                    if cp <= 16:
                        emit_score_pair(cp)
                    dp = cp - 3
                    if 0 <= dp <= 16:
                        # +2-chunk lookahead on bank init; bounded so the
                        # bufs=2 rotation stays behind group finishes
                        gmax = min(NSUB - 1, 2 * dp + 1 + 2) // 4
                        for g in range(gmax + 1):
                            ensure_group(g)
                        for c in (2 * dp, 2 * dp + 1):
                            emit_densamp_chunk(c)
                            if c >= 5 and (c - 5) % 4 == 0:
                                emit_finish_group((c - 5) // 4)

    return nc


def _prep_inputs(x, W1, b1, W2, b2, Wout, bout):
    m2g, d2h = _build_masks()
    bf = ml_dtypes.bfloat16

    blob_bf = np.zeros((128, BLOBW), dtype=bf)

    def put_bf(col, arr):
        arr = np.asarray(arr).astype(bf)
        blob_bf[:arr.shape[0], col:col + arr.shape[1]] = arr

    def put_f32(col, arr):
        arr = np.ascontiguousarray(np.asarray(arr, np.float32))
        v = arr.view(np.uint16).reshape(arr.shape[0], -1)
        blob_bf[:arr.shape[0], col:col + v.shape[1]] = v.view(bf)

    put_bf(O_W2T, np.ascontiguousarray(W2.T))
    # Wout/2 absorbs the softmax 1/2 left over from the 2*sigmoid gate
    put_bf(O_WOT, np.ascontiguousarray(np.asarray(Wout, np.float32).T * 0.5))
    put_bf(O_M2G, m2g)
    put_bf(O_IDENT, np.eye(128, dtype=np.float32))
    put_bf(O_ONESB, np.ones((128, 128), np.float32))
    put_f32(O_B1, np.asarray(b1, np.float32).reshape(CH, 1))
    put_f32(O_B2H, (0.5 * np.asarray(b2, np.float32)).reshape(CH, 1))
    put_f32(O_BOUT, np.ascontiguousarray(
        np.asarray(bout, np.float32).reshape(2, CH).T))
    put_bf(O_D2H, d2h)

    common = {
        "W1T": np.ascontiguousarray(W1.T).astype(bf),
        "blob": blob_bf,
    }
    in_maps = []
    for i in range(N):
        m = dict(common)
        m["x"] = np.ascontiguousarray(
            np.asarray(x[i], np.float32).reshape(CIN, HW)).astype(bf)
        in_maps.append(m)
    return in_maps


_CACHED = {}


def kernel(x, W1, b1, W2, b2, Wout, bout):
    if "nc" not in _CACHED:
        nc = build_nc()
        nc.finalize()
        _CACHED["nc"] = nc
    nc = _CACHED["nc"]
    in_maps = _prep_inputs(x, W1, b1, W2, b2, Wout, bout)
    res = run_bass_kernel_spmd(nc, in_maps, core_ids=list(range(N)))
    out = np.stack([np.asarray(res.results[i]["out"], dtype=np.float32)
                    .reshape(CIN, H, W) for i in range(N)])
    return out


# revision 24
# speedup vs baseline: 1.2532x; 1.2532x over previous
"""Trainium2 Bass kernel for nn_Attribution (sparse local-window attention).

Data-parallel over batch n=8 -> one batch element per NeuronCore.

Per-core computation (c_in=256, ch=128, 64x64 image):
    h    = W1 @ x + b1
    corr = 5x5 local window correlation of h (zero padded), /sqrt(128)
    attn = softmax over the 25 window entries
    samp = sum_k attn_k * shift_k(h)
    gate = sigmoid(relu(W2 @ h + b2)) = 0.5 + 0.5*relu(tanh((z+b2)/2))
    out  = Wout @ (gate * samp) + bout

Layout: positions flattened row-major with 2 zero-pad rows top/bottom
(68 rows x 64 = 4352 positions = 34 chunks of 128).  Scores "born
transposed" (keys of chunk c on partitions, queries on free axis).
Out-of-window entries killed by a {0,1} mask after exp; out-of-image x
neighbors accounted by denominator correction D (exp(0)=1 each in the
zero-padded reference).

This version keeps the PE stream minimal and the output phase fused into
the chunk pipeline:
  - den/samp PSUM banks are pre-zeroed by memsets on DVE/GPSIMD (idle
    engines), so every den/samp matmul is a plain accumulate and the 16
    pre-zero PE matmuls of the previous version are gone,
  - den uses an all-ones [128,128] stationary so the column sums land
    broadcast across all partitions: the reciprocal is computed full-width
    on DVE and the PE partition-broadcast matmuls are gone,
  - each group's normalize + output conv + bias + store is emitted as soon
    as its denominator closes, so output DMA streams during the chunk
    pipeline instead of draining at the end,
  - evacuations are spread: ACT does only exp/tanh, DVE does mask/recip/
    normalize/conv1-bias, GPSIMD does transpose-evac/gate/attr/out-bias,
  - input rides in 8 fat x DMAs + 2 weight + 2 blob DMAs over 4 queues.
"""
import sys

sys.path.insert(0, "/opt/trn_rl_repo")

import numpy as np
import ml_dtypes

import concourse.bass as bass
import concourse.mybir as mybir
import concourse.tile as tile
from concourse import bacc
from concourse.bass_utils import run_bass_kernel_spmd

F32 = mybir.dt.float32
BF16 = mybir.dt.bfloat16
AF = mybir.ActivationFunctionType
ALU = mybir.AluOpType

N, CIN, CH, H, W = 8, 256, 128, 64, 64
HW = H * W                      # 4096
RAD = 2
KROWS = H + 2 * RAD             # 68 padded rows
PADPOS = KROWS * W              # 4352
NCHUNK = PADPOS // 128          # 34 key chunks (2 rows each)
NSUB = H // 2                   # 32 query subs (128 queries each)
NGRP = NSUB // 4                # 8 groups of 4 subs (one PSUM bank each)
SCALE = 1.0 / np.sqrt(np.float32(CH))

# ---- const blob layout (bf16 [128, BLOBW]) ----
O_W2T = 0            # [128,128]
O_WOT = 128          # [128,256]
O_M2G = 384          # maskC2g [128,896]
O_IDENT = 1280       # [128,128]
O_ONESB = 1408       # [128,128] all-ones
O_B1 = 1536          # [128,1] f32 (2 bf16 cols)
O_B2H = 1538         # [128,1] f32
O_BOUT = 1540        # [128,2] f32 (4 bf16 cols)
O_D2H = 1544         # [128,512] bf16: D(q) tiled, identical rows
BLOBW = 2056


def _build_masks():
    """maskC2g: (128, 896) {0,1} bf16 = maskC | zeros(128) | maskC.
    maskC col 128*a+q is key (chunk c, pos p) vs query q of sub s=c-2+a:
    valid iff |2-2a + p//64 - q//64| <= 2 and |p%64 - q%64| <= 2."""
    m = np.zeros((128, 384), dtype=np.float32)
    for a in range(3):
        for p in range(128):
            for q in range(128):
                dy = 2 - 2 * a + p // 64 - q // 64
                if abs(dy) <= RAD and abs(p % 64 - q % 64) <= RAD:
                    m[p, 128 * a + q] = 1.0
    m2g = np.concatenate([m, np.zeros((128, 128), np.float32), m], axis=1)

    cnt = np.array([sum(1 for dx in range(-RAD, RAD + 1) if not 0 <= qx + dx < W)
                    for qx in range(W)], dtype=np.float32)
    drow = 5.0 * np.concatenate([cnt, cnt])                 # (128,) D
    d2h = np.tile(np.tile(drow, 4)[None, :], (128, 1))      # (128,512)
    return m2g.astype(ml_dtypes.bfloat16), d2h.astype(ml_dtypes.bfloat16)


def _chunk_parts(c):
    """den/samp MM parts for chunk c: (g, s_lo, s_hi) sub-ranges split at
    4-sub PSUM bank boundaries.  Banks are pre-zeroed by memsets, so every
    part is a plain accumulate."""
    smin, smax = max(0, c - 2), min(NSUB - 1, c)
    parts = []
    for g in range(smin // 4, smax // 4 + 1):
        parts.append((g, max(smin, 4 * g), min(smax, 4 * g + 3)))
    return parts


def build_nc(repeat=1, sim_safe=False):
    nc = bacc.Bacc("TRN2", target_bir_lowering=False, debug=False, num_devices=8)

    x_d = nc.declare_dram_parameter("x", [CIN, HW], BF16, isOutput=False)
    w1t_d = nc.declare_dram_parameter("W1T", [CIN, CH], BF16, isOutput=False)
    blob_d = nc.declare_dram_parameter("blob", [128, BLOBW], BF16, isOutput=False)
    out_d = nc.declare_dram_parameter("out", [CIN, HW], BF16, isOutput=True)

    with tile.TileContext(nc) as tc:
        with (
            tc.tile_pool(name="per", bufs=1) as per,
            tc.tile_pool(name="smp", bufs=8) as smp,
            tc.tile_pool(name="otp", bufs=4) as otp,
            tc.tile_pool(name="pA", bufs=2, space="PSUM") as pA,   # 2x[128,1024] f32
            tc.tile_pool(name="pB", bufs=2, space="PSUM") as pB,   # 2x[128,512] f32 samp
            tc.tile_pool(name="pD", bufs=2, space="PSUM") as pD,   # 2x[128,512] f32 den
        ):
            blobw = per.tile([128, 2 * CH], BF16, tag="blobw")
            blob = per.tile([128, BLOBW], BF16, tag="blob")
            xall = per.tile([128, 2 * HW], BF16, tag="xall")
            hpad = per.tile([128, PADPOS], BF16, tag="hpad")
            hT = per.tile([128, PADPOS], BF16, tag="hT")
            attnm = per.tile([128, NCHUNK * 512], BF16, tag="attnm")
            Pg = per.tile([128, HW], BF16, tag="Pg")
            attr = per.tile([128, HW], BF16, tag="attr")

            w1t0 = blobw[:, 0:CH]
            w1t1 = blobw[:, CH:2 * CH]
            w2t = blob[:, O_W2T:O_W2T + 128]
            wot = blob[:, O_WOT:O_WOT + 256]
            maskC2g = blob[:, O_M2G:O_M2G + 896]
            maskC = blob[:, O_M2G:O_M2G + 384]
            ident = blob[:, O_IDENT:O_IDENT + 128]
            onesb = blob[:, O_ONESB:O_ONESB + 128]
            b1 = blob[:, O_B1:O_B1 + 2].bitcast(F32)
            b2h = blob[:, O_B2H:O_B2H + 2].bitcast(F32)
            bout0 = blob[:, O_BOUT:O_BOUT + 2].bitcast(F32)
            bout1 = blob[:, O_BOUT + 2:O_BOUT + 4].bitcast(F32)
            d2h = blob[:, O_D2H:O_D2H + 512]

            # --- input DMAs over 4 issue queues.  Per queue: the weight /
            # blob piece that queue owns, then x blocks in consumption
            # order.  Each dma_start is packetized across all 16 HW DMA
            # engines, so few fat transfers saturate the ~250GB/s link.
            def xdma(eng, half, u):
                src = x_d[128 * half:128 * (half + 1), 1024 * u:1024 * (u + 1)]
                eng.dma_start(
                    xall[:, HW * half + 1024 * u:HW * half + 1024 * (u + 1)], src)

            nc.sync.dma_start(blobw[:, 0:CH], w1t_d[0:128, :])
            nc.scalar.dma_start(blobw[:, CH:2 * CH], w1t_d[128:256, :])
            # first half-block finer so the first conv matmul starts early
            nc.sync.dma_start(xall[:, 0:512], x_d[0:128, 0:512])
            nc.scalar.dma_start(xall[:, HW:HW + 512], x_d[128:256, 0:512])
            nc.gpsimd.dma_start(blob[:, O_IDENT:BLOBW], blob_d[:, O_IDENT:BLOBW])
            nc.sync.dma_start(xall[:, 512:1024], x_d[0:128, 512:1024])
            nc.scalar.dma_start(xall[:, HW + 512:HW + 1024], x_d[128:256, 512:1024])
            xdma(nc.gpsimd, 0, 1)
            xdma(nc.sync, 1, 1)
            nc.scalar.dma_start(blob[:, 0:O_IDENT], blob_d[:, 0:O_IDENT])
            xdma(nc.gpsimd, 0, 2)
            xdma(nc.sync, 1, 2)
            xdma(nc.scalar, 0, 3)
            xdma(nc.gpsimd, 1, 3)

            # pad chunks (0 and 33) are identically zero
            nc.vector.memset(hpad[:, 0:128], 0.0)
            nc.vector.memset(hpad[:, PADPOS - 128:PADPOS], 0.0)
            nc.gpsimd.memset(hT[:, 0:128], 0.0)
            nc.gpsimd.memset(hT[:, PADPOS - 128:PADPOS], 0.0)

            for _rep in range(repeat):
                # ---- P1: conv1 + transposes + conv2, PE kept streaming.
                def emit_transp_group(u):
                    pt = pA.tile([128, 1024], BF16, tag="pa", name=f"pt{u}")
                    for k in range(8):
                        c = 8 * u + 1 + k
                        nc.tensor.transpose(pt[:, 128 * k:128 * (k + 1)],
                                            hpad[:, 128 * c:128 * (c + 1)],
                                            ident)
                    nc.scalar.copy(hT[:, 128 * (8 * u + 1):128 * (8 * u + 9)],
                                   pt[:])

                def emit_conv2(b):
                    pz = pB.tile([128, 512], F32, tag="pb", name=f"pz{b}")
                    nc.tensor.matmul(pz[:], w2t,
                                     hpad[:, 128 + 512 * b:128 + 512 * (b + 1)],
                                     start=True, stop=True)
                    tg = smp.tile([128, 512], BF16, tag="tg")
                    nc.scalar.activation(tg[:], pz[:], AF.Tanh, scale=0.5, bias=b2h)
                    nc.vector.tensor_scalar(
                        out=Pg[:, 512 * b:512 * (b + 1)], in0=tg[:],
                        scalar1=0.0, scalar2=1.0, op0=ALU.max, op1=ALU.add)

                for u in range(4):
                    cvt = pA.tile([128, 1024], F32, tag="pa", name=f"cv{u}")
                    for h2 in range(2):
                        dst = cvt[:, 512 * h2:512 * (h2 + 1)]
                        cs = slice(1024 * u + 512 * h2, 1024 * u + 512 * (h2 + 1))
                        cs2 = slice(HW + cs.start, HW + cs.stop)
                        nc.tensor.matmul(dst, w1t0, xall[:, cs], start=True, stop=False)
                        nc.tensor.matmul(dst, w1t1, xall[:, cs2], start=False, stop=True)
                    nc.vector.tensor_scalar(
                        out=hpad[:, 128 + 1024 * u:128 + 1024 * (u + 1)],
                        in0=cvt[:], scalar1=b1, scalar2=None, op0=ALU.add)
                    if u >= 1:
                        emit_transp_group(u - 1)
                        emit_conv2(2 * (u - 1))
                        emit_conv2(2 * (u - 1) + 1)
                emit_transp_group(3)
                emit_conv2(6)
                emit_conv2(7)

                # ---- P2: chunk pipeline: scores/exp/mask with den+samp
                # matmuls trailing two pairs behind on the PE; each group's
                # recip/normalize/output conv/store fires as soon as its
                # denominator closes.
                deng = {}
                sampg = {}

                def ensure_group(g):
                    if g in deng or g >= NGRP:
                        return
                    deng[g] = pD.tile([128, 512], F32, tag="pd", name=f"dn{g}")
                    sampg[g] = pB.tile([128, 512], F32, tag="pb", name=f"sp{g}")
                    # den preset = D(q) (out-of-image correction), samp = 0;
                    # all den/samp matmuls then accumulate on top.
                    nc.scalar.copy(deng[g][:], d2h)
                    nc.vector.memset(sampg[g][:], 0.0)

                def emit_score_pair(cp):
                    sc = pA.tile([128, 1024], F32, tag="pa", name=f"sc{cp}")
                    spans = []
                    for ci in range(2):
                        c = 2 * cp + ci
                        lo, hi = max(0, c - 2), min(NSUB - 1, c)
                        alo = lo - (c - 2)
                        spans.append((alo, alo + hi - lo + 1))
                        nc.tensor.matmul(
                            sc[:, 512 * ci + 128 * alo:512 * ci + 128 * (alo + hi - lo + 1)],
                            hpad[:, 128 * c:128 * (c + 1)],
                            hpad[:, 128 * (lo + 1):128 * (hi + 2)],
                            start=True, stop=True)
                    meng = nc.vector
                    if spans == [(0, 3), (0, 3)]:
                        asl = attnm[:, 1024 * cp:1024 * cp + 896]
                        nc.scalar.activation(asl, sc[:, 0:896], AF.Exp,
                                             scale=float(SCALE))
                        meng.tensor_tensor(out=asl, in0=asl,
                                           in1=maskC2g, op=ALU.mult)
                    else:
                        for ci, (a0, a1) in enumerate(spans):
                            ss = slice(512 * ci + 128 * a0, 512 * ci + 128 * a1)
                            asl = attnm[:, 1024 * cp + ss.start:1024 * cp + ss.stop]
                            nc.scalar.activation(asl, sc[:, ss], AF.Exp,
                                                 scale=float(SCALE))
                            nc.vector.tensor_tensor(
                                out=asl, in0=asl,
                                in1=maskC[:, 128 * a0:128 * a1], op=ALU.mult)

                def emit_densamp_chunk(c):
                    parts = _chunk_parts(c)
                    for g, s, e in parts:
                        aa = s - (c - 2)
                        rhs = attnm[:, 512 * c + 128 * aa:512 * c + 128 * (aa + e - s + 1)]
                        nc.tensor.matmul(
                            deng[g][:, 128 * (s - 4 * g):128 * (e + 1 - 4 * g)],
                            onesb, rhs, start=False, stop=False,
                            skip_group_check=True)
                    for g, s, e in parts:
                        aa = s - (c - 2)
                        nc.tensor.matmul(
                            sampg[g][:, 128 * (s - 4 * g):128 * (e + 1 - 4 * g)],
                            hT[:, 128 * c:128 * (c + 1)],
                            attnm[:, 512 * c + 128 * aa:512 * c + 128 * (aa + e - s + 1)],
                            start=False, stop=False, skip_group_check=True)

                def emit_finish_group(g):
                    gsl = slice(512 * g, 512 * (g + 1))
                    # z = 1 / (den + D): den banks were preset with D, so a
                    # plain full-width reciprocal does it (den is broadcast
                    # across partitions by the ones stationary).  The
                    # softmax 1/2 vs gate 2x cancels via Wout/2 on host.
                    z = smp.tile([128, 512], F32, tag="z", name=f"z{g}")
                    nc.vector.reciprocal_approx_fast(z[:], deng[g][:])
                    # attr = (samp * Pg) * z ; the z-normalize is all-SBUF so
                    # it can ride on gpsimd
                    nc.vector.tensor_tensor(out=attr[:, gsl], in0=sampg[g][:],
                                            in1=Pg[:, gsl], op=ALU.mult)
                    nc.vector.tensor_tensor(out=attr[:, gsl], in0=attr[:, gsl],
                                            in1=z[:], op=ALU.mult)
                    # output conv + bias + store for this group
                    po = pA.tile([128, 1024], F32, tag="pa", name=f"po{g}")
                    nc.tensor.matmul(po[:, 0:512], wot[:, 0:128], attr[:, gsl],
                                     start=True, stop=True)
                    nc.tensor.matmul(po[:, 512:1024], wot[:, 128:256], attr[:, gsl],
                                     start=True, stop=True)
                    ot = otp.tile([128, 1024], BF16, tag="ot")
                    nc.scalar.activation(ot[:, 0:512], po[:, 0:512],
                                         AF.Identity, bias=bout0, scale=1.0)
                    nc.scalar.activation(ot[:, 512:1024], po[:, 512:1024],
                                         AF.Identity, bias=bout1, scale=1.0)
                    nsp = 2 if g == NGRP - 1 else 1
                    for oc in range(2):
                        osl = slice(512 * oc, 512 * (oc + 1))
                        for j in range(nsp):
                            w = 512 // nsp
                            qcs = slice(512 * g + w * j, 512 * g + w * (j + 1))
                            ts = slice(osl.start + w * j, osl.start + w * (j + 1))
                            nc.sync.dma_start(out_d[128 * oc:128 * (oc + 1), qcs],
                                              ot[:, ts])

                ensure_group(0)
                ensure_group(1)
                for cp in range(20):
                    if cp <= 16:
                        emit_score_pair(cp)
                    dp = cp - 3
                    if 0 <= dp <= 16:
                        # +2-chunk lookahead on bank init; bounded so the
                        # bufs=2 rotation stays behind group finishes
                        gmax = min(NSUB - 1, 2 * dp + 1 + 2) // 4
                        for g in range(gmax + 1):
                            ensure_group(g)
                        for c in (2 * dp, 2 * dp + 1):
                            emit_densamp_chunk(c)
                            if c >= 5 and (c - 5) % 4 == 0:
                                emit_finish_group((c - 5) // 4)

    return nc


def _prep_inputs(x, W1, b1, W2, b2, Wout, bout):
    m2g, d2h = _build_masks()
    bf = ml_dtypes.bfloat16

    blob_bf = np.zeros((128, BLOBW), dtype=bf)

    def put_bf(col, arr):
        arr = np.asarray(arr).astype(bf)
        blob_bf[:arr.shape[0], col:col + arr.shape[1]] = arr

    def put_f32(col, arr):
        arr = np.ascontiguousarray(np.asarray(arr, np.float32))
        v = arr.view(np.uint16).reshape(arr.shape[0], -1)
        blob_bf[:arr.shape[0], col:col + v.shape[1]] = v.view(bf)

    put_bf(O_W2T, np.ascontiguousarray(W2.T))
    # Wout/2 absorbs the softmax 1/2 left over from the 2*sigmoid gate
    put_bf(O_WOT, np.ascontiguousarray(np.asarray(Wout, np.float32).T * 0.5))
    put_bf(O_M2G, m2g)
    put_bf(O_IDENT, np.eye(128, dtype=np.float32))
    put_bf(O_ONESB, np.ones((128, 128), np.float32))
    put_f32(O_B1, np.asarray(b1, np.float32).reshape(CH, 1))
    put_f32(O_B2H, (0.5 * np.asarray(b2, np.float32)).reshape(CH, 1))
    put_f32(O_BOUT, np.ascontiguousarray(
        np.asarray(bout, np.float32).reshape(2, CH).T))
    put_bf(O_D2H, d2h)

    common = {
        "W1T": np.ascontiguousarray(W1.T).astype(bf),
        "blob": blob_bf,
    }
    in_maps = []
    for i in range(N):
        m = dict(common)
        m["x"] = np.ascontiguousarray(
            np.asarray(x[i], np.float32).reshape(CIN, HW)).astype(bf)
        in_maps.append(m)
    return in_maps


_CACHED = {}


def kernel(x, W1, b1, W2, b2, Wout, bout):
    if "nc" not in _CACHED:
        nc = build_nc()
        nc.finalize()
        _CACHED["nc"] = nc
    nc = _CACHED["nc"]
    in_maps = _prep_inputs(x, W1, b1, W2, b2, Wout, bout)
    res = run_bass_kernel_spmd(nc, in_maps, core_ids=list(range(N)))
    out = np.stack([np.asarray(res.results[i]["out"], dtype=np.float32)
                    .reshape(CIN, H, W) for i in range(N)])
    return out


# revision 26
# speedup vs baseline: 1.3084x; 1.0441x over previous
"""Trainium2 Bass kernel for nn_Attribution (sparse local-window attention).

Data-parallel over batch n=8 -> one batch element per NeuronCore.

Per-core computation (c_in=256, ch=128, 64x64 image):
    h    = W1 @ x + b1
    corr = 5x5 local window correlation of h (zero padded), /sqrt(128)
    attn = softmax over the 25 window entries
    samp = sum_k attn_k * shift_k(h)
    gate = sigmoid(relu(W2 @ h + b2)) = 0.5 + 0.5*relu(tanh((z+b2)/2))
    out  = Wout @ (gate * samp) + bout

Layout: positions flattened row-major with 2 zero-pad rows top/bottom
(68 rows x 64 = 4352 positions = 34 chunks of 128).  Scores "born
transposed" (keys of chunk c on partitions, queries on free axis).
Out-of-window entries killed by a {0,1} mask after exp; out-of-image x
neighbors accounted by denominator correction D (exp(0)=1 each in the
zero-padded reference).

This version keeps the PE stream minimal and the output phase fused into
the chunk pipeline:
  - den/samp PSUM banks are pre-zeroed by memsets on DVE/GPSIMD (idle
    engines), so every den/samp matmul is a plain accumulate and the 16
    pre-zero PE matmuls of the previous version are gone,
  - den uses an all-ones [128,128] stationary so the column sums land
    broadcast across all partitions: the reciprocal is computed full-width
    on DVE and the PE partition-broadcast matmuls are gone,
  - each group's normalize + output conv + bias + store is emitted as soon
    as its denominator closes, so output DMA streams during the chunk
    pipeline instead of draining at the end,
  - evacuations are spread: ACT does only exp/tanh, DVE does mask/recip/
    normalize/conv1-bias, GPSIMD does transpose-evac/gate/attr/out-bias,
  - input rides in 8 fat x DMAs + 2 weight + 2 blob DMAs over 4 queues.
"""
import sys

sys.path.insert(0, "/opt/trn_rl_repo")

import numpy as np
import ml_dtypes

import concourse.bass as bass
import concourse.mybir as mybir
import concourse.tile as tile
from concourse import bacc
from concourse.bass_utils import run_bass_kernel_spmd

F32 = mybir.dt.float32
BF16 = mybir.dt.bfloat16
AF = mybir.ActivationFunctionType
ALU = mybir.AluOpType

N, CIN, CH, H, W = 8, 256, 128, 64, 64
HW = H * W                      # 4096
RAD = 2
KROWS = H + 2 * RAD             # 68 padded rows
PADPOS = KROWS * W              # 4352
NCHUNK = PADPOS // 128          # 34 key chunks (2 rows each)
NSUB = H // 2                   # 32 query subs (128 queries each)
NGRP = NSUB // 4                # 8 groups of 4 subs (one PSUM bank each)
SCALE = 1.0 / np.sqrt(np.float32(CH))

# ---- const blob layout (bf16 [128, BLOBW]) ----
O_W2T = 0            # [128,128]
O_WOT = 128          # [128,256]
O_M2G = 384          # maskC2g [128,896]
O_IDENT = 1280       # [128,128]
O_ONESB = 1408       # [128,128] all-ones
O_B1 = 1536          # [128,1] f32 (2 bf16 cols)
O_B2H = 1538         # [128,1] f32
O_BOUT = 1540        # [128,2] f32 (4 bf16 cols)
O_D2H = 1544         # [128,512] bf16: D(q) tiled, identical rows
BLOBW = 2056


def _build_masks():
    """maskC2g: (128, 896) {0,1} bf16 = maskC | zeros(128) | maskC.
    maskC col 128*a+q is key (chunk c, pos p) vs query q of sub s=c-2+a:
    valid iff |2-2a + p//64 - q//64| <= 2 and |p%64 - q%64| <= 2."""
    m = np.zeros((128, 384), dtype=np.float32)
    for a in range(3):
        for p in range(128):
            for q in range(128):
                dy = 2 - 2 * a + p // 64 - q // 64
                if abs(dy) <= RAD and abs(p % 64 - q % 64) <= RAD:
                    m[p, 128 * a + q] = 1.0
    m2g = np.concatenate([m, np.zeros((128, 128), np.float32), m], axis=1)

    cnt = np.array([sum(1 for dx in range(-RAD, RAD + 1) if not 0 <= qx + dx < W)
                    for qx in range(W)], dtype=np.float32)
    drow = 5.0 * np.concatenate([cnt, cnt])                 # (128,) D
    d2h = np.tile(np.tile(drow, 4)[None, :], (128, 1))      # (128,512)
    return m2g.astype(ml_dtypes.bfloat16), d2h.astype(ml_dtypes.bfloat16)


def _chunk_parts(c):
    """den/samp MM parts for chunk c: (g, s_lo, s_hi) sub-ranges split at
    4-sub PSUM bank boundaries.  Banks are pre-zeroed by memsets, so every
    part is a plain accumulate."""
    smin, smax = max(0, c - 2), min(NSUB - 1, c)
    parts = []
    for g in range(smin // 4, smax // 4 + 1):
        parts.append((g, max(smin, 4 * g), min(smax, 4 * g + 3)))
    return parts


def build_nc(repeat=1, sim_safe=False):
    nc = bacc.Bacc("TRN2", target_bir_lowering=False, debug=False, num_devices=8)

    x_d = nc.declare_dram_parameter("x", [CIN, HW], BF16, isOutput=False)
    w1t_d = nc.declare_dram_parameter("W1T", [CIN, CH], BF16, isOutput=False)
    blob_d = nc.declare_dram_parameter("blob", [128, BLOBW], BF16, isOutput=False)
    out_d = nc.declare_dram_parameter("out", [CIN, HW], BF16, isOutput=True)

    with tile.TileContext(nc) as tc:
        with (
            tc.tile_pool(name="per", bufs=1) as per,
            tc.tile_pool(name="smp", bufs=8) as smp,
            tc.tile_pool(name="otp", bufs=4) as otp,
            tc.tile_pool(name="pA", bufs=2, space="PSUM") as pA,   # 2x[128,1024] f32
            tc.tile_pool(name="pB", bufs=2, space="PSUM") as pB,   # 2x[128,512] f32 samp
            tc.tile_pool(name="pD", bufs=2, space="PSUM") as pD,   # 2x[128,512] f32 den
        ):
            blobw = per.tile([128, 2 * CH], BF16, tag="blobw")
            blob = per.tile([128, BLOBW], BF16, tag="blob")
            xall = per.tile([128, 2 * HW], BF16, tag="xall")
            hpad = per.tile([128, PADPOS], BF16, tag="hpad")
            hT = per.tile([128, PADPOS], BF16, tag="hT")
            attnm = per.tile([128, NCHUNK * 512], BF16, tag="attnm")
            Pg = per.tile([128, HW], BF16, tag="Pg")
            attr = per.tile([128, HW], BF16, tag="attr")

            w1t0 = blobw[:, 0:CH]
            w1t1 = blobw[:, CH:2 * CH]
            w2t = blob[:, O_W2T:O_W2T + 128]
            wot = blob[:, O_WOT:O_WOT + 256]
            maskC2g = blob[:, O_M2G:O_M2G + 896]
            maskC = blob[:, O_M2G:O_M2G + 384]
            ident = blob[:, O_IDENT:O_IDENT + 128]
            onesb = blob[:, O_ONESB:O_ONESB + 128]
            b1 = blob[:, O_B1:O_B1 + 2].bitcast(F32)
            b2h = blob[:, O_B2H:O_B2H + 2].bitcast(F32)
            bout0 = blob[:, O_BOUT:O_BOUT + 2].bitcast(F32)
            bout1 = blob[:, O_BOUT + 2:O_BOUT + 4].bitcast(F32)
            d2h = blob[:, O_D2H:O_D2H + 512]

            # --- input DMAs over 4 issue queues.  Per queue: the weight /
            # blob piece that queue owns, then x blocks in consumption
            # order.  Each dma_start is packetized across all 16 HW DMA
            # engines, so few fat transfers saturate the ~250GB/s link.
            def xdma(eng, half, u):
                src = x_d[128 * half:128 * (half + 1), 1024 * u:1024 * (u + 1)]
                eng.dma_start(
                    xall[:, HW * half + 1024 * u:HW * half + 1024 * (u + 1)], src)

            nc.sync.dma_start(blobw[:, 0:CH], w1t_d[0:128, :])
            nc.scalar.dma_start(blobw[:, CH:2 * CH], w1t_d[128:256, :])
            # first half-block finer so the first conv matmul starts early
            nc.sync.dma_start(xall[:, 0:512], x_d[0:128, 0:512])
            nc.scalar.dma_start(xall[:, HW:HW + 512], x_d[128:256, 0:512])
            nc.gpsimd.dma_start(blob[:, O_IDENT:BLOBW], blob_d[:, O_IDENT:BLOBW])
            nc.sync.dma_start(xall[:, 512:1024], x_d[0:128, 512:1024])
            nc.scalar.dma_start(xall[:, HW + 512:HW + 1024], x_d[128:256, 512:1024])
            xdma(nc.gpsimd, 0, 1)
            xdma(nc.sync, 1, 1)
            nc.scalar.dma_start(blob[:, 0:O_IDENT], blob_d[:, 0:O_IDENT])
            xdma(nc.gpsimd, 0, 2)
            xdma(nc.sync, 1, 2)
            xdma(nc.scalar, 0, 3)
            xdma(nc.gpsimd, 1, 3)

            # pad chunks (0 and 33) are identically zero
            nc.vector.memset(hpad[:, 0:128], 0.0)
            nc.vector.memset(hpad[:, PADPOS - 128:PADPOS], 0.0)
            nc.gpsimd.memset(hT[:, 0:128], 0.0)
            nc.gpsimd.memset(hT[:, PADPOS - 128:PADPOS], 0.0)

            for _rep in range(repeat):
                # ---- P1: conv1 + transposes + conv2, PE kept streaming.
                def emit_transp_group(u):
                    pt = pA.tile([128, 1024], BF16, tag="pa", name=f"pt{u}")
                    for k in range(8):
                        c = 8 * u + 1 + k
                        nc.tensor.transpose(pt[:, 128 * k:128 * (k + 1)],
                                            hpad[:, 128 * c:128 * (c + 1)],
                                            ident)
                    nc.scalar.copy(hT[:, 128 * (8 * u + 1):128 * (8 * u + 9)],
                                   pt[:])

                def emit_conv2(b):
                    pz = pB.tile([128, 512], F32, tag="pb", name=f"pz{b}")
                    nc.tensor.matmul(pz[:], w2t,
                                     hpad[:, 128 + 512 * b:128 + 512 * (b + 1)],
                                     start=True, stop=True)
                    tg = smp.tile([128, 512], BF16, tag="tg")
                    nc.scalar.activation(tg[:], pz[:], AF.Tanh, scale=0.5, bias=b2h)
                    nc.vector.tensor_scalar(
                        out=Pg[:, 512 * b:512 * (b + 1)], in0=tg[:],
                        scalar1=0.0, scalar2=1.0, op0=ALU.max, op1=ALU.add)

                def emit_conv_u(u):
                    cvt = pA.tile([128, 1024], F32, tag="pa", name=f"cv{u}")
                    for h2 in range(2):
                        dst = cvt[:, 512 * h2:512 * (h2 + 1)]
                        cs = slice(1024 * u + 512 * h2, 1024 * u + 512 * (h2 + 1))
                        cs2 = slice(HW + cs.start, HW + cs.stop)
                        nc.tensor.matmul(dst, w1t0, xall[:, cs], start=True, stop=False)
                        nc.tensor.matmul(dst, w1t1, xall[:, cs2], start=False, stop=True)
                    nc.vector.tensor_scalar(
                        out=hpad[:, 128 + 1024 * u:128 + 1024 * (u + 1)],
                        in0=cvt[:], scalar1=b1, scalar2=None, op0=ALU.add)

                deng = {}
                sampg = {}

                def ensure_group(g):
                    if g in deng or g >= NGRP:
                        return
                    deng[g] = pD.tile([128, 512], F32, tag="pd", name=f"dn{g}")
                    sampg[g] = pB.tile([128, 512], F32, tag="pb", name=f"sp{g}")
                    # den preset = D(q) (out-of-image correction), samp = 0;
                    # all den/samp matmuls then accumulate on top.
                    nc.scalar.copy(deng[g][:], d2h)
                    nc.vector.memset(sampg[g][:], 0.0)

                def emit_score_pair(cp):
                    sc = pA.tile([128, 1024], F32, tag="pa", name=f"sc{cp}")
                    spans = []
                    for ci in range(2):
                        c = 2 * cp + ci
                        lo, hi = max(0, c - 2), min(NSUB - 1, c)
                        alo = lo - (c - 2)
                        spans.append((alo, alo + hi - lo + 1))
                        nc.tensor.matmul(
                            sc[:, 512 * ci + 128 * alo:512 * ci + 128 * (alo + hi - lo + 1)],
                            hpad[:, 128 * c:128 * (c + 1)],
                            hpad[:, 128 * (lo + 1):128 * (hi + 2)],
                            start=True, stop=True)
                    meng = nc.vector
                    if spans == [(0, 3), (0, 3)]:
                        asl = attnm[:, 1024 * cp:1024 * cp + 896]
                        nc.scalar.activation(asl, sc[:, 0:896], AF.Exp,
                                             scale=float(SCALE))
                        meng.tensor_tensor(out=asl, in0=asl,
                                           in1=maskC2g, op=ALU.mult)
                    else:
                        for ci, (a0, a1) in enumerate(spans):
                            ss = slice(512 * ci + 128 * a0, 512 * ci + 128 * a1)
                            asl = attnm[:, 1024 * cp + ss.start:1024 * cp + ss.stop]
                            nc.scalar.activation(asl, sc[:, ss], AF.Exp,
                                                 scale=float(SCALE))
                            nc.vector.tensor_tensor(
                                out=asl, in0=asl,
                                in1=maskC[:, 128 * a0:128 * a1], op=ALU.mult)

                def emit_densamp_chunk(c):
                    parts = _chunk_parts(c)
                    for g, s, e in parts:
                        aa = s - (c - 2)
                        rhs = attnm[:, 512 * c + 128 * aa:512 * c + 128 * (aa + e - s + 1)]
                        nc.tensor.matmul(
                            deng[g][:, 128 * (s - 4 * g):128 * (e + 1 - 4 * g)],
                            onesb, rhs, start=False, stop=False,
                            skip_group_check=True)
                    for g, s, e in parts:
                        aa = s - (c - 2)
                        nc.tensor.matmul(
                            sampg[g][:, 128 * (s - 4 * g):128 * (e + 1 - 4 * g)],
                            hT[:, 128 * c:128 * (c + 1)],
                            attnm[:, 512 * c + 128 * aa:512 * c + 128 * (aa + e - s + 1)],
                            start=False, stop=False, skip_group_check=True)

                def emit_finish_a(g):
                    gsl = slice(512 * g, 512 * (g + 1))
                    # z = 1 / (den + D): den banks were preset with D, so a
                    # plain full-width reciprocal does it (den is broadcast
                    # across partitions by the ones stationary).  The
                    # softmax 1/2 vs gate 2x cancels via Wout/2 on host.
                    z = smp.tile([128, 512], F32, tag="z", name=f"z{g}")
                    nc.vector.reciprocal_approx_fast(z[:], deng[g][:])
                    # attr = (samp * Pg) * z
                    nc.vector.tensor_tensor(out=attr[:, gsl], in0=sampg[g][:],
                                            in1=Pg[:, gsl], op=ALU.mult)
                    nc.vector.tensor_tensor(out=attr[:, gsl], in0=attr[:, gsl],
                                            in1=z[:], op=ALU.mult)

                def emit_finish_b(g):
                    # output conv + bias + store, one ds-pair after finish_a
                    # so the PE never waits on the DVE normalize chain
                    gsl = slice(512 * g, 512 * (g + 1))
                    po = pA.tile([128, 1024], F32, tag="pa", name=f"po{g}")
                    nc.tensor.matmul(po[:, 0:512], wot[:, 0:128], attr[:, gsl],
                                     start=True, stop=True)
                    nc.tensor.matmul(po[:, 512:1024], wot[:, 128:256], attr[:, gsl],
                                     start=True, stop=True)
                    ot = otp.tile([128, 1024], BF16, tag="ot")
                    nc.scalar.activation(ot[:, 0:512], po[:, 0:512],
                                         AF.Identity, bias=bout0, scale=1.0)
                    nc.scalar.activation(ot[:, 512:1024], po[:, 512:1024],
                                         AF.Identity, bias=bout1, scale=1.0)
                    nsp = 2 if g == NGRP - 1 else 1
                    for oc in range(2):
                        osl = slice(512 * oc, 512 * (oc + 1))
                        for j in range(nsp):
                            w = 512 // nsp
                            qcs = slice(512 * g + w * j, 512 * g + w * (j + 1))
                            ts = slice(osl.start + w * j, osl.start + w * (j + 1))
                            nc.sync.dma_start(out_d[128 * oc:128 * (oc + 1), qcs],
                                              ot[:, ts])

                fb_queue = []

                def emit_dsp(dp):
                    while fb_queue and fb_queue[0][1] < dp:
                        emit_finish_b(fb_queue.pop(0)[0])
                    gmax = min(NSUB - 1, 2 * dp + 3) // 4
                    for g in range(gmax + 1):
                        ensure_group(g)
                    for c in (2 * dp, 2 * dp + 1):
                        emit_densamp_chunk(c)
                        if c >= 5 and (c - 5) % 4 == 0:
                            emit_finish_a((c - 5) // 4)
                            fb_queue.append(((c - 5) // 4, dp))

                # ---- interleaved schedule: conv1/transpose/conv2 (gated on
                # the x stream) with the score pipeline filling DMA-wait
                # gaps, then the chunk pipeline with den/samp trailing the
                # scores and group outputs streaming as denominators close.
                emit_conv_u(0)
                emit_conv_u(1)
                emit_transp_group(0)
                emit_conv2(0)
                emit_conv2(1)
                emit_score_pair(0)
                emit_score_pair(1)
                emit_score_pair(2)
                ensure_group(0)
                ensure_group(1)
                emit_conv_u(2)
                emit_transp_group(1)
                emit_conv2(2)
                emit_conv2(3)
                emit_score_pair(3)
                emit_score_pair(4)
                emit_conv_u(3)
                emit_transp_group(2)
                emit_conv2(4)
                emit_conv2(5)
                emit_score_pair(5)
                emit_dsp(0)
                emit_score_pair(6)
                emit_dsp(1)
                emit_transp_group(3)
                emit_conv2(6)
                emit_conv2(7)
                dp = 2
                for cp in range(7, 20):
                    if cp <= 16:
                        emit_score_pair(cp)
                    nds = 2 if cp <= 8 else 1
                    for _ in range(nds):
                        if dp <= 16:
                            emit_dsp(dp)
                            dp += 1
                while fb_queue:
                    emit_finish_b(fb_queue.pop(0)[0])

    return nc


def _prep_inputs(x, W1, b1, W2, b2, Wout, bout):
    m2g, d2h = _build_masks()
    bf = ml_dtypes.bfloat16

    blob_bf = np.zeros((128, BLOBW), dtype=bf)

    def put_bf(col, arr):
        arr = np.asarray(arr).astype(bf)
        blob_bf[:arr.shape[0], col:col + arr.shape[1]] = arr

    def put_f32(col, arr):
        arr = np.ascontiguousarray(np.asarray(arr, np.float32))
        v = arr.view(np.uint16).reshape(arr.shape[0], -1)
        blob_bf[:arr.shape[0], col:col + v.shape[1]] = v.view(bf)

    put_bf(O_W2T, np.ascontiguousarray(W2.T))
    # Wout/2 absorbs the softmax 1/2 left over from the 2*sigmoid gate
    put_bf(O_WOT, np.ascontiguousarray(np.asarray(Wout, np.float32).T * 0.5))
    put_bf(O_M2G, m2g)
    put_bf(O_IDENT, np.eye(128, dtype=np.float32))
    put_bf(O_ONESB, np.ones((128, 128), np.float32))
    put_f32(O_B1, np.asarray(b1, np.float32).reshape(CH, 1))
    put_f32(O_B2H, (0.5 * np.asarray(b2, np.float32)).reshape(CH, 1))
    put_f32(O_BOUT, np.ascontiguousarray(
        np.asarray(bout, np.float32).reshape(2, CH).T))
    put_bf(O_D2H, d2h)

    common = {
        "W1T": np.ascontiguousarray(W1.T).astype(bf),
        "blob": blob_bf,
    }
    in_maps = []
    for i in range(N):
        m = dict(common)
        m["x"] = np.ascontiguousarray(
            np.asarray(x[i], np.float32).reshape(CIN, HW)).astype(bf)
        in_maps.append(m)
    return in_maps


_CACHED = {}


def kernel(x, W1, b1, W2, b2, Wout, bout):
    if "nc" not in _CACHED:
        nc = build_nc()
        nc.finalize()
        _CACHED["nc"] = nc
    nc = _CACHED["nc"]
    in_maps = _prep_inputs(x, W1, b1, W2, b2, Wout, bout)
    res = run_bass_kernel_spmd(nc, in_maps, core_ids=list(range(N)))
    out = np.stack([np.asarray(res.results[i]["out"], dtype=np.float32)
                    .reshape(CIN, H, W) for i in range(N)])
    return out


# revision 27
# speedup vs baseline: 1.3195x; 1.0085x over previous
"""Trainium2 Bass kernel for nn_Attribution (sparse local-window attention).

Data-parallel over batch n=8 -> one batch element per NeuronCore.

Per-core computation (c_in=256, ch=128, 64x64 image):
    h    = W1 @ x + b1
    corr = 5x5 local window correlation of h (zero padded), /sqrt(128)
    attn = softmax over the 25 window entries
    samp = sum_k attn_k * shift_k(h)
    gate = sigmoid(relu(W2 @ h + b2)) = 0.5 + 0.5*relu(tanh((z+b2)/2))
    out  = Wout @ (gate * samp) + bout

Layout: positions flattened row-major with 2 zero-pad rows top/bottom
(68 rows x 64 = 4352 positions = 34 chunks of 128).  Scores "born
transposed" (keys of chunk c on partitions, queries on free axis).
Out-of-window entries killed by a {0,1} mask after exp; out-of-image x
neighbors accounted by denominator correction D (exp(0)=1 each in the
zero-padded reference).

This version keeps the PE stream minimal and the output phase fused into
the chunk pipeline:
  - den/samp PSUM banks are pre-zeroed by memsets on DVE/GPSIMD (idle
    engines), so every den/samp matmul is a plain accumulate and the 16
    pre-zero PE matmuls of the previous version are gone,
  - den uses an all-ones [128,128] stationary so the column sums land
    broadcast across all partitions: the reciprocal is computed full-width
    on DVE and the PE partition-broadcast matmuls are gone,
  - each group's normalize + output conv + bias + store is emitted as soon
    as its denominator closes, so output DMA streams during the chunk
    pipeline instead of draining at the end,
  - evacuations are spread: ACT does only exp/tanh, DVE does mask/recip/
    normalize/conv1-bias, GPSIMD does transpose-evac/gate/attr/out-bias,
  - input rides in 8 fat x DMAs + 2 weight + 2 blob DMAs over 4 queues.
"""
import sys

sys.path.insert(0, "/opt/trn_rl_repo")

import numpy as np
import ml_dtypes

import concourse.bass as bass
import concourse.mybir as mybir
import concourse.tile as tile
from concourse import bacc
from concourse.bass_utils import run_bass_kernel_spmd

F32 = mybir.dt.float32
BF16 = mybir.dt.bfloat16
AF = mybir.ActivationFunctionType
ALU = mybir.AluOpType

N, CIN, CH, H, W = 8, 256, 128, 64, 64
HW = H * W                      # 4096
RAD = 2
KROWS = H + 2 * RAD             # 68 padded rows
PADPOS = KROWS * W              # 4352
NCHUNK = PADPOS // 128          # 34 key chunks (2 rows each)
NSUB = H // 2                   # 32 query subs (128 queries each)
NGRP = NSUB // 4                # 8 groups of 4 subs (one PSUM bank each)
SCALE = 1.0 / np.sqrt(np.float32(CH))

# ---- const blob layout (bf16 [128, BLOBW]) ----
O_W2T = 0            # [128,128]
O_WOT = 128          # [128,256]
O_M2G = 384          # maskC2g [128,896]
O_IDENT = 1280       # [128,128]
O_ONESB = 1408       # [128,128] all-ones
O_B1 = 1536          # [128,1] f32 (2 bf16 cols)
O_B2H = 1538         # [128,1] f32
O_BOUT = 1540        # [128,2] f32 (4 bf16 cols)
O_D2H = 1544         # [128,512] bf16: D(q) tiled, identical rows
BLOBW = 2056


def _build_masks():
    """maskC2g: (128, 896) {0,1} bf16 = maskC | zeros(128) | maskC.
    maskC col 128*a+q is key (chunk c, pos p) vs query q of sub s=c-2+a:
    valid iff |2-2a + p//64 - q//64| <= 2 and |p%64 - q%64| <= 2."""
    m = np.zeros((128, 384), dtype=np.float32)
    for a in range(3):
        for p in range(128):
            for q in range(128):
                dy = 2 - 2 * a + p // 64 - q // 64
                if abs(dy) <= RAD and abs(p % 64 - q % 64) <= RAD:
                    m[p, 128 * a + q] = 1.0
    m2g = np.concatenate([m, np.zeros((128, 128), np.float32), m], axis=1)

    cnt = np.array([sum(1 for dx in range(-RAD, RAD + 1) if not 0 <= qx + dx < W)
                    for qx in range(W)], dtype=np.float32)
    drow = 5.0 * np.concatenate([cnt, cnt])                 # (128,) D
    d2h = np.tile(np.tile(drow, 4)[None, :], (128, 1))      # (128,512)
    return m2g.astype(ml_dtypes.bfloat16), d2h.astype(ml_dtypes.bfloat16)


def _chunk_parts(c):
    """den/samp MM parts for chunk c: (g, s_lo, s_hi) sub-ranges split at
    4-sub PSUM bank boundaries.  Banks are pre-zeroed by memsets, so every
    part is a plain accumulate."""
    smin, smax = max(0, c - 2), min(NSUB - 1, c)
    parts = []
    for g in range(smin // 4, smax // 4 + 1):
        parts.append((g, max(smin, 4 * g), min(smax, 4 * g + 3)))
    return parts


def build_nc(repeat=1, sim_safe=False):
    nc = bacc.Bacc("TRN2", target_bir_lowering=False, debug=False, num_devices=8)

    x_d = nc.declare_dram_parameter("x", [CIN, HW], BF16, isOutput=False)
    w1t_d = nc.declare_dram_parameter("W1T", [CIN, CH], BF16, isOutput=False)
    blob_d = nc.declare_dram_parameter("blob", [128, BLOBW], BF16, isOutput=False)
    out_d = nc.declare_dram_parameter("out", [CIN, HW], BF16, isOutput=True)

    with tile.TileContext(nc) as tc:
        with (
            tc.tile_pool(name="per", bufs=1) as per,
            tc.tile_pool(name="smp", bufs=8) as smp,
            tc.tile_pool(name="otp", bufs=4) as otp,
            tc.tile_pool(name="pA", bufs=2, space="PSUM") as pA,   # 2x[128,1024] f32
            tc.tile_pool(name="pB", bufs=2, space="PSUM") as pB,   # 2x[128,512] f32 samp
            tc.tile_pool(name="pD", bufs=2, space="PSUM") as pD,   # 2x[128,512] f32 den
        ):
            blobw = per.tile([128, 2 * CH], BF16, tag="blobw")
            blob = per.tile([128, BLOBW], BF16, tag="blob")
            xall = per.tile([128, 2 * HW], BF16, tag="xall")
            hpad = per.tile([128, PADPOS], BF16, tag="hpad")
            hT = per.tile([128, PADPOS], BF16, tag="hT")
            attnm = per.tile([128, NCHUNK * 512], BF16, tag="attnm")
            Pg = per.tile([128, HW], BF16, tag="Pg")
            attr = per.tile([128, HW], BF16, tag="attr")

            w1t0 = blobw[:, 0:CH]
            w1t1 = blobw[:, CH:2 * CH]
            w2t = blob[:, O_W2T:O_W2T + 128]
            wot = blob[:, O_WOT:O_WOT + 256]
            maskC2g = blob[:, O_M2G:O_M2G + 896]
            maskC = blob[:, O_M2G:O_M2G + 384]
            ident = blob[:, O_IDENT:O_IDENT + 128]
            onesb = blob[:, O_ONESB:O_ONESB + 128]
            b1 = blob[:, O_B1:O_B1 + 2].bitcast(F32)
            b2h = blob[:, O_B2H:O_B2H + 2].bitcast(F32)
            bout0 = blob[:, O_BOUT:O_BOUT + 2].bitcast(F32)
            bout1 = blob[:, O_BOUT + 2:O_BOUT + 4].bitcast(F32)
            d2h = blob[:, O_D2H:O_D2H + 512]

            # --- input DMAs over 4 issue queues.  Per queue: the weight /
            # blob piece that queue owns, then x blocks in consumption
            # order.  Each dma_start is packetized across all 16 HW DMA
            # engines, so few fat transfers saturate the ~250GB/s link.
            def xdma(eng, half, u):
                src = x_d[128 * half:128 * (half + 1), 1024 * u:1024 * (u + 1)]
                eng.dma_start(
                    xall[:, HW * half + 1024 * u:HW * half + 1024 * (u + 1)], src)

            nc.sync.dma_start(blobw[:, 0:CH], w1t_d[0:128, :])
            nc.scalar.dma_start(blobw[:, CH:2 * CH], w1t_d[128:256, :])
            # first half-block finer so the first conv matmul starts early
            nc.sync.dma_start(xall[:, 0:512], x_d[0:128, 0:512])
            nc.scalar.dma_start(xall[:, HW:HW + 512], x_d[128:256, 0:512])
            nc.gpsimd.dma_start(blob[:, O_IDENT:BLOBW], blob_d[:, O_IDENT:BLOBW])
            nc.sync.dma_start(xall[:, 512:1024], x_d[0:128, 512:1024])
            nc.scalar.dma_start(xall[:, HW + 512:HW + 1024], x_d[128:256, 512:1024])
            xdma(nc.gpsimd, 0, 1)
            xdma(nc.sync, 1, 1)
            nc.scalar.dma_start(blob[:, 0:O_IDENT], blob_d[:, 0:O_IDENT])
            xdma(nc.gpsimd, 0, 2)
            xdma(nc.sync, 1, 2)
            xdma(nc.scalar, 0, 3)
            xdma(nc.gpsimd, 1, 3)

            # pad chunks (0 and 33) are identically zero
            nc.vector.memset(hpad[:, 0:128], 0.0)
            nc.vector.memset(hpad[:, PADPOS - 128:PADPOS], 0.0)
            nc.gpsimd.memset(hT[:, 0:128], 0.0)
            nc.gpsimd.memset(hT[:, PADPOS - 128:PADPOS], 0.0)

            # sanitize the two den PSUM buffers with full-width start=True
            # matmuls (values irrelevant; the D preset overwrites them).  On
            # the first execution the banks can hold pending-zero state that
            # would otherwise discard the ACT-written preset at the first
            # accumulate.  Runs during the input-DMA wait, so it is free.
            for j in range(2):
                dnI = pD.tile([128, 512], F32, tag="pd", name=f"dnI{j}")
                nc.tensor.matmul(dnI[:], hpad[:, 0:128], hpad[:, 0:512],
                                 start=True, stop=True)

            for _rep in range(repeat):
                # ---- P1: conv1 + transposes + conv2, PE kept streaming.
                def emit_transp_group(u):
                    pt = pA.tile([128, 1024], BF16, tag="pa", name=f"pt{u}")
                    for k in range(8):
                        c = 8 * u + 1 + k
                        nc.tensor.transpose(pt[:, 128 * k:128 * (k + 1)],
                                            hpad[:, 128 * c:128 * (c + 1)],
                                            ident)
                    nc.scalar.copy(hT[:, 128 * (8 * u + 1):128 * (8 * u + 9)],
                                   pt[:])

                def emit_conv2(b):
                    pz = pB.tile([128, 512], F32, tag="pb", name=f"pz{b}")
                    nc.tensor.matmul(pz[:], w2t,
                                     hpad[:, 128 + 512 * b:128 + 512 * (b + 1)],
                                     start=True, stop=True)
                    tg = smp.tile([128, 512], BF16, tag="tg")
                    nc.scalar.activation(tg[:], pz[:], AF.Tanh, scale=0.5, bias=b2h)
                    nc.vector.tensor_scalar(
                        out=Pg[:, 512 * b:512 * (b + 1)], in0=tg[:],
                        scalar1=0.0, scalar2=1.0, op0=ALU.max, op1=ALU.add)

                def emit_conv_u(u):
                    cvt = pA.tile([128, 1024], F32, tag="pa", name=f"cv{u}")
                    for h2 in range(2):
                        dst = cvt[:, 512 * h2:512 * (h2 + 1)]
                        cs = slice(1024 * u + 512 * h2, 1024 * u + 512 * (h2 + 1))
                        cs2 = slice(HW + cs.start, HW + cs.stop)
                        nc.tensor.matmul(dst, w1t0, xall[:, cs], start=True, stop=False)
                        nc.tensor.matmul(dst, w1t1, xall[:, cs2], start=False, stop=True)
                    nc.vector.tensor_scalar(
                        out=hpad[:, 128 + 1024 * u:128 + 1024 * (u + 1)],
                        in0=cvt[:], scalar1=b1, scalar2=None, op0=ALU.add)

                deng = {}
                sampg = {}

                def ensure_group(g):
                    if g in deng or g >= NGRP:
                        return
                    deng[g] = pD.tile([128, 512], F32, tag="pd", name=f"dn{g}")
                    sampg[g] = pB.tile([128, 512], F32, tag="pb", name=f"sp{g}")
                    # den preset = D(q) (out-of-image correction), samp = 0;
                    # all den/samp matmuls then accumulate on top.
                    nc.scalar.copy(deng[g][:], d2h)
                    nc.vector.memset(sampg[g][:], 0.0)

                def emit_score_pair(cp):
                    sc = pA.tile([128, 1024], F32, tag="pa", name=f"sc{cp}")
                    spans = []
                    for ci in range(2):
                        c = 2 * cp + ci
                        lo, hi = max(0, c - 2), min(NSUB - 1, c)
                        alo = lo - (c - 2)
                        spans.append((alo, alo + hi - lo + 1))
                        nc.tensor.matmul(
                            sc[:, 512 * ci + 128 * alo:512 * ci + 128 * (alo + hi - lo + 1)],
                            hpad[:, 128 * c:128 * (c + 1)],
                            hpad[:, 128 * (lo + 1):128 * (hi + 2)],
                            start=True, stop=True)
                    meng = nc.vector
                    if spans == [(0, 3), (0, 3)]:
                        asl = attnm[:, 1024 * cp:1024 * cp + 896]
                        nc.scalar.activation(asl, sc[:, 0:896], AF.Exp,
                                             scale=float(SCALE))
                        meng.tensor_tensor(out=asl, in0=asl,
                                           in1=maskC2g, op=ALU.mult)
                    else:
                        for ci, (a0, a1) in enumerate(spans):
                            ss = slice(512 * ci + 128 * a0, 512 * ci + 128 * a1)
                            asl = attnm[:, 1024 * cp + ss.start:1024 * cp + ss.stop]
                            nc.scalar.activation(asl, sc[:, ss], AF.Exp,
                                                 scale=float(SCALE))
                            nc.vector.tensor_tensor(
                                out=asl, in0=asl,
                                in1=maskC[:, 128 * a0:128 * a1], op=ALU.mult)

                def emit_densamp_chunk(c):
                    parts = _chunk_parts(c)
                    for g, s, e in parts:
                        aa = s - (c - 2)
                        rhs = attnm[:, 512 * c + 128 * aa:512 * c + 128 * (aa + e - s + 1)]
                        nc.tensor.matmul(
                            deng[g][:, 128 * (s - 4 * g):128 * (e + 1 - 4 * g)],
                            onesb, rhs, start=False, stop=False,
                            skip_group_check=True)
                    for g, s, e in parts:
                        aa = s - (c - 2)
                        nc.tensor.matmul(
                            sampg[g][:, 128 * (s - 4 * g):128 * (e + 1 - 4 * g)],
                            hT[:, 128 * c:128 * (c + 1)],
                            attnm[:, 512 * c + 128 * aa:512 * c + 128 * (aa + e - s + 1)],
                            start=False, stop=False, skip_group_check=True)

                def emit_finish_a(g):
                    gsl = slice(512 * g, 512 * (g + 1))
                    # z = 1 / (den + D): den banks were preset with D, so a
                    # plain full-width reciprocal does it (den is broadcast
                    # across partitions by the ones stationary).  The
                    # softmax 1/2 vs gate 2x cancels via Wout/2 on host.
                    z = smp.tile([128, 512], F32, tag="z", name=f"z{g}")
                    nc.vector.reciprocal_approx_fast(z[:], deng[g][:])
                    # attr = (samp * Pg) * z
                    nc.vector.tensor_tensor(out=attr[:, gsl], in0=sampg[g][:],
                                            in1=Pg[:, gsl], op=ALU.mult)
                    nc.vector.tensor_tensor(out=attr[:, gsl], in0=attr[:, gsl],
                                            in1=z[:], op=ALU.mult)

                def emit_finish_b(g):
                    # output conv + bias + store, one ds-pair after finish_a
                    # so the PE never waits on the DVE normalize chain
                    gsl = slice(512 * g, 512 * (g + 1))
                    po = pA.tile([128, 1024], F32, tag="pa", name=f"po{g}")
                    nc.tensor.matmul(po[:, 0:512], wot[:, 0:128], attr[:, gsl],
                                     start=True, stop=True)
                    nc.tensor.matmul(po[:, 512:1024], wot[:, 128:256], attr[:, gsl],
                                     start=True, stop=True)
                    ot = otp.tile([128, 1024], BF16, tag="ot")
                    nc.scalar.activation(ot[:, 0:512], po[:, 0:512],
                                         AF.Identity, bias=bout0, scale=1.0)
                    nc.scalar.activation(ot[:, 512:1024], po[:, 512:1024],
                                         AF.Identity, bias=bout1, scale=1.0)
                    nsp = 2 if g == NGRP - 1 else 1
                    for oc in range(2):
                        osl = slice(512 * oc, 512 * (oc + 1))
                        for j in range(nsp):
                            w = 512 // nsp
                            qcs = slice(512 * g + w * j, 512 * g + w * (j + 1))
                            ts = slice(osl.start + w * j, osl.start + w * (j + 1))
                            nc.sync.dma_start(out_d[128 * oc:128 * (oc + 1), qcs],
                                              ot[:, ts])

                fb_queue = []

                def emit_dsp(dp):
                    while fb_queue and fb_queue[0][1] < dp:
                        emit_finish_b(fb_queue.pop(0)[0])
                    gmax = min(NSUB - 1, 2 * dp + 3) // 4
                    for g in range(gmax + 1):
                        ensure_group(g)
                    for c in (2 * dp, 2 * dp + 1):
                        emit_densamp_chunk(c)
                        if c >= 5 and (c - 5) % 4 == 0:
                            emit_finish_a((c - 5) // 4)
                            fb_queue.append(((c - 5) // 4, dp))

                # ---- interleaved schedule: conv1/transpose/conv2 (gated on
                # the x stream) with the score pipeline filling DMA-wait
                # gaps, then the chunk pipeline with den/samp trailing the
                # scores and group outputs streaming as denominators close.
                emit_conv_u(0)
                emit_conv_u(1)
                emit_transp_group(0)
                emit_conv2(0)
                emit_conv2(1)
                emit_score_pair(0)
                emit_score_pair(1)
                emit_score_pair(2)
                ensure_group(0)
                ensure_group(1)
                emit_conv_u(2)
                emit_transp_group(1)
                emit_conv2(2)
                emit_conv2(3)
                emit_score_pair(3)
                emit_score_pair(4)
                emit_conv_u(3)
                emit_transp_group(2)
                emit_conv2(4)
                emit_conv2(5)
                emit_score_pair(5)
                emit_dsp(0)
                emit_score_pair(6)
                emit_dsp(1)
                emit_transp_group(3)
                emit_conv2(6)
                emit_conv2(7)
                dp = 2
                for cp in range(7, 20):
                    if cp <= 16:
                        emit_score_pair(cp)
                    nds = 2 if cp <= 8 else 1
                    for _ in range(nds):
                        if dp <= 16:
                            emit_dsp(dp)
                            dp += 1
                while fb_queue:
                    emit_finish_b(fb_queue.pop(0)[0])

    return nc


def _prep_inputs(x, W1, b1, W2, b2, Wout, bout):
    m2g, d2h = _build_masks()
    bf = ml_dtypes.bfloat16

    blob_bf = np.zeros((128, BLOBW), dtype=bf)

    def put_bf(col, arr):
        arr = np.asarray(arr).astype(bf)
        blob_bf[:arr.shape[0], col:col + arr.shape[1]] = arr

    def put_f32(col, arr):
        arr = np.ascontiguousarray(np.asarray(arr, np.float32))
        v = arr.view(np.uint16).reshape(arr.shape[0], -1)
        blob_bf[:arr.shape[0], col:col + v.shape[1]] = v.view(bf)

    put_bf(O_W2T, np.ascontiguousarray(W2.T))
    # Wout/2 absorbs the softmax 1/2 left over from the 2*sigmoid gate
    put_bf(O_WOT, np.ascontiguousarray(np.asarray(Wout, np.float32).T * 0.5))
    put_bf(O_M2G, m2g)
    put_bf(O_IDENT, np.eye(128, dtype=np.float32))
    put_bf(O_ONESB, np.ones((128, 128), np.float32))
    put_f32(O_B1, np.asarray(b1, np.float32).reshape(CH, 1))
    put_f32(O_B2H, (0.5 * np.asarray(b2, np.float32)).reshape(CH, 1))
    put_f32(O_BOUT, np.ascontiguousarray(
        np.asarray(bout, np.float32).reshape(2, CH).T))
    put_bf(O_D2H, d2h)

    common = {
        "W1T": np.ascontiguousarray(W1.T).astype(bf),
        "blob": blob_bf,
    }
    in_maps = []
    for i in range(N):
        m = dict(common)
        m["x"] = np.ascontiguousarray(
            np.asarray(x[i], np.float32).reshape(CIN, HW)).astype(bf)
        in_maps.append(m)
    return in_maps


_CACHED = {}


def kernel(x, W1, b1, W2, b2, Wout, bout):
    if "nc" not in _CACHED:
        nc = build_nc()
        nc.finalize()
        _CACHED["nc"] = nc
    nc = _CACHED["nc"]
    in_maps = _prep_inputs(x, W1, b1, W2, b2, Wout, bout)
    res = run_bass_kernel_spmd(nc, in_maps, core_ids=list(range(N)))
    out = np.stack([np.asarray(res.results[i]["out"], dtype=np.float32)
                    .reshape(CIN, H, W) for i in range(N)])
    return out


# revision 29
# speedup vs baseline: 1.3390x; 1.0147x over previous
"""Trainium2 Bass kernel for nn_Attribution (sparse local-window attention).

Data-parallel over batch n=8 -> one batch element per NeuronCore.

Per-core computation (c_in=256, ch=128, 64x64 image):
    h    = W1 @ x + b1
    corr = 5x5 local window correlation of h (zero padded), /sqrt(128)
    attn = softmax over the 25 window entries
    samp = sum_k attn_k * shift_k(h)
    gate = sigmoid(relu(W2 @ h + b2)) = 0.5 + 0.5*relu(tanh((z+b2)/2))
    out  = Wout @ (gate * samp) + bout

Layout: positions flattened row-major with 2 zero-pad rows top/bottom
(68 rows x 64 = 4352 positions = 34 chunks of 128).  Scores "born
transposed" (keys of chunk c on partitions, queries on free axis).
Out-of-window entries killed by a {0,1} mask after exp; out-of-image x
neighbors accounted by denominator correction D (exp(0)=1 each in the
zero-padded reference).

This version keeps the PE stream minimal and the output phase fused into
the chunk pipeline:
  - den/samp PSUM banks are pre-zeroed by memsets on DVE/GPSIMD (idle
    engines), so every den/samp matmul is a plain accumulate and the 16
    pre-zero PE matmuls of the previous version are gone,
  - den uses an all-ones [128,128] stationary so the column sums land
    broadcast across all partitions: the reciprocal is computed full-width
    on DVE and the PE partition-broadcast matmuls are gone,
  - each group's normalize + output conv + bias + store is emitted as soon
    as its denominator closes, so output DMA streams during the chunk
    pipeline instead of draining at the end,
  - evacuations are spread: ACT does only exp/tanh, DVE does mask/recip/
    normalize/conv1-bias, GPSIMD does transpose-evac/gate/attr/out-bias,
  - input rides in 8 fat x DMAs + 2 weight + 2 blob DMAs over 4 queues.
"""
import sys

sys.path.insert(0, "/opt/trn_rl_repo")

import numpy as np
import ml_dtypes

import concourse.bass as bass
import concourse.mybir as mybir
import concourse.tile as tile
from concourse import bacc
from concourse.bass_utils import run_bass_kernel_spmd

F32 = mybir.dt.float32
BF16 = mybir.dt.bfloat16
AF = mybir.ActivationFunctionType
ALU = mybir.AluOpType

N, CIN, CH, H, W = 8, 256, 128, 64, 64
HW = H * W                      # 4096
RAD = 2
KROWS = H + 2 * RAD             # 68 padded rows
PADPOS = KROWS * W              # 4352
NCHUNK = PADPOS // 128          # 34 key chunks (2 rows each)
NSUB = H // 2                   # 32 query subs (128 queries each)
NGRP = NSUB // 4                # 8 groups of 4 subs (one PSUM bank each)
SCALE = 1.0 / np.sqrt(np.float32(CH))

# ---- const blob layout (bf16 [128, BLOBW]) ----
O_W2T = 0            # [128,128]
O_WOT = 128          # [128,256]
O_M2G = 384          # maskC2g [128,896]
O_IDENT = 1280       # [128,128]
O_ONESB = 1408       # [128,128] all-ones
O_B1 = 1536          # [128,1] f32 (2 bf16 cols)
O_B2H = 1538         # [128,1] f32
O_BOUT = 1540        # [128,2] f32 (4 bf16 cols)
O_D2H = 1544         # [128,512] bf16: D(q) tiled, identical rows
BLOBW = 2056


def _build_masks():
    """maskC2g: (128, 896) {0,1} bf16 = maskC | zeros(128) | maskC.
    maskC col 128*a+q is key (chunk c, pos p) vs query q of sub s=c-2+a:
    valid iff |2-2a + p//64 - q//64| <= 2 and |p%64 - q%64| <= 2."""
    m = np.zeros((128, 384), dtype=np.float32)
    for a in range(3):
        for p in range(128):
            for q in range(128):
                dy = 2 - 2 * a + p // 64 - q // 64
                if abs(dy) <= RAD and abs(p % 64 - q % 64) <= RAD:
                    m[p, 128 * a + q] = 1.0
    m2g = np.concatenate([m, np.zeros((128, 128), np.float32), m], axis=1)

    cnt = np.array([sum(1 for dx in range(-RAD, RAD + 1) if not 0 <= qx + dx < W)
                    for qx in range(W)], dtype=np.float32)
    drow = 5.0 * np.concatenate([cnt, cnt])                 # (128,) D
    d2h = np.tile(np.tile(drow, 4)[None, :], (128, 1))      # (128,512)
    return m2g.astype(ml_dtypes.bfloat16), d2h.astype(ml_dtypes.bfloat16)


def _chunk_parts(c):
    """den/samp MM parts for chunk c: (g, s_lo, s_hi) sub-ranges split at
    4-sub PSUM bank boundaries.  Banks are pre-zeroed by memsets, so every
    part is a plain accumulate."""
    smin, smax = max(0, c - 2), min(NSUB - 1, c)
    parts = []
    for g in range(smin // 4, smax // 4 + 1):
        parts.append((g, max(smin, 4 * g), min(smax, 4 * g + 3)))
    return parts


def build_nc(repeat=1, sim_safe=False):
    nc = bacc.Bacc("TRN2", target_bir_lowering=False, debug=False, num_devices=8)

    x_d = nc.declare_dram_parameter("x", [CIN, HW], BF16, isOutput=False)
    w1t_d = nc.declare_dram_parameter("W1T", [CIN, CH], BF16, isOutput=False)
    blob_d = nc.declare_dram_parameter("blob", [128, BLOBW], BF16, isOutput=False)
    out_d = nc.declare_dram_parameter("out", [CIN, HW], BF16, isOutput=True)

    with tile.TileContext(nc) as tc:
        with (
            tc.tile_pool(name="per", bufs=1) as per,
            tc.tile_pool(name="smp", bufs=8) as smp,
            tc.tile_pool(name="otp", bufs=4) as otp,
            tc.tile_pool(name="pA", bufs=2, space="PSUM") as pA,   # 2x[128,1024] f32
            tc.tile_pool(name="pB", bufs=2, space="PSUM") as pB,   # 2x[128,512] f32 samp
            tc.tile_pool(name="pD", bufs=2, space="PSUM") as pD,   # 2x[128,512] f32 den
        ):
            blobw = per.tile([128, 2 * CH], BF16, tag="blobw")
            blob = per.tile([128, BLOBW], BF16, tag="blob")
            xall = per.tile([128, 2 * HW], BF16, tag="xall")
            hpad = per.tile([128, PADPOS], BF16, tag="hpad")
            hT = per.tile([128, PADPOS], BF16, tag="hT")
            attnm = per.tile([128, NCHUNK * 512], BF16, tag="attnm")
            Pg = per.tile([128, HW], BF16, tag="Pg")
            attr = per.tile([128, HW], BF16, tag="attr")

            w1t0 = blobw[:, 0:CH]
            w1t1 = blobw[:, CH:2 * CH]
            w2t = blob[:, O_W2T:O_W2T + 128]
            wot = blob[:, O_WOT:O_WOT + 256]
            maskC2g = blob[:, O_M2G:O_M2G + 896]
            maskC = blob[:, O_M2G:O_M2G + 384]
            ident = blob[:, O_IDENT:O_IDENT + 128]
            onesb = blob[:, O_ONESB:O_ONESB + 128]
            b1 = blob[:, O_B1:O_B1 + 2].bitcast(F32)
            b2h = blob[:, O_B2H:O_B2H + 2].bitcast(F32)
            bout0 = blob[:, O_BOUT:O_BOUT + 2].bitcast(F32)
            bout1 = blob[:, O_BOUT + 2:O_BOUT + 4].bitcast(F32)
            d2h = blob[:, O_D2H:O_D2H + 512]

            # --- input DMAs over 4 issue queues.  Per queue: the weight /
            # blob piece that queue owns, then x blocks in consumption
            # order.  Each dma_start is packetized across all 16 HW DMA
            # engines, so few fat transfers saturate the ~250GB/s link.
            def xdma(eng, half, u):
                src = x_d[128 * half:128 * (half + 1), 1024 * u:1024 * (u + 1)]
                eng.dma_start(
                    xall[:, HW * half + 1024 * u:HW * half + 1024 * (u + 1)], src)

            nc.sync.dma_start(blobw[:, 0:CH], w1t_d[0:128, :])
            nc.scalar.dma_start(blobw[:, CH:2 * CH], w1t_d[128:256, :])
            # first half-block finer so the first conv matmul starts early
            nc.sync.dma_start(xall[:, 0:512], x_d[0:128, 0:512])
            nc.scalar.dma_start(xall[:, HW:HW + 512], x_d[128:256, 0:512])
            nc.gpsimd.dma_start(blob[:, O_IDENT:BLOBW], blob_d[:, O_IDENT:BLOBW])
            nc.sync.dma_start(xall[:, 512:1024], x_d[0:128, 512:1024])
            nc.scalar.dma_start(xall[:, HW + 512:HW + 1024], x_d[128:256, 512:1024])
            xdma(nc.gpsimd, 0, 1)
            xdma(nc.sync, 1, 1)
            nc.scalar.dma_start(blob[:, 0:O_IDENT], blob_d[:, 0:O_IDENT])
            xdma(nc.gpsimd, 0, 2)
            xdma(nc.sync, 1, 2)
            xdma(nc.scalar, 0, 3)
            xdma(nc.gpsimd, 1, 3)

            # pad chunks (0 and 33) are identically zero
            nc.vector.memset(hpad[:, 0:128], 0.0)
            nc.vector.memset(hpad[:, PADPOS - 128:PADPOS], 0.0)
            nc.gpsimd.memset(hT[:, 0:128], 0.0)
            nc.gpsimd.memset(hT[:, PADPOS - 128:PADPOS], 0.0)

            # sanitize the two den PSUM buffers with full-width start=True
            # matmuls (values irrelevant; the D preset overwrites them).  On
            # the first execution the banks can hold pending-zero state that
            # would otherwise discard the ACT-written preset at the first
            # accumulate.  Runs during the input-DMA wait, so it is free.
            for j in range(2):
                dnI = pD.tile([128, 512], F32, tag="pd", name=f"dnI{j}")
                nc.tensor.matmul(dnI[:], hpad[:, 0:128], hpad[:, 0:512],
                                 start=True, stop=True)

            for _rep in range(repeat):
                # ---- P1: conv1 + transposes + conv2, PE kept streaming.
                def emit_transp_group(u):
                    pt = pA.tile([128, 1024], BF16, tag="pa", name=f"pt{u}")
                    for k in range(8):
                        c = 8 * u + 1 + k
                        nc.tensor.transpose(pt[:, 128 * k:128 * (k + 1)],
                                            hpad[:, 128 * c:128 * (c + 1)],
                                            ident)
                    nc.scalar.copy(hT[:, 128 * (8 * u + 1):128 * (8 * u + 9)],
                                   pt[:])

                def emit_conv2(b):
                    pz = pB.tile([128, 512], F32, tag="pb", name=f"pz{b}")
                    nc.tensor.matmul(pz[:], w2t,
                                     hpad[:, 128 + 512 * b:128 + 512 * (b + 1)],
                                     start=True, stop=True)
                    tg = smp.tile([128, 512], BF16, tag="tg")
                    nc.scalar.activation(tg[:], pz[:], AF.Tanh, scale=0.5, bias=b2h)
                    nc.vector.tensor_scalar(
                        out=Pg[:, 512 * b:512 * (b + 1)], in0=tg[:],
                        scalar1=0.0, scalar2=1.0, op0=ALU.max, op1=ALU.add)

                def emit_conv_u(u):
                    cvt = pA.tile([128, 1024], F32, tag="pa", name=f"cv{u}")
                    for h2 in range(2):
                        dst = cvt[:, 512 * h2:512 * (h2 + 1)]
                        cs = slice(1024 * u + 512 * h2, 1024 * u + 512 * (h2 + 1))
                        cs2 = slice(HW + cs.start, HW + cs.stop)
                        nc.tensor.matmul(dst, w1t0, xall[:, cs], start=True, stop=False)
                        nc.tensor.matmul(dst, w1t1, xall[:, cs2], start=False, stop=True)
                    nc.vector.tensor_scalar(
                        out=hpad[:, 128 + 1024 * u:128 + 1024 * (u + 1)],
                        in0=cvt[:], scalar1=b1, scalar2=None, op0=ALU.add)

                deng = {}
                sampg = {}

                def ensure_group(g):
                    if g in deng or g >= NGRP:
                        return
                    deng[g] = pD.tile([128, 512], F32, tag="pd", name=f"dn{g}")
                    sampg[g] = pB.tile([128, 512], F32, tag="pb", name=f"sp{g}")
                    # den preset = D(q) (out-of-image correction), samp = 0;
                    # all den/samp matmuls then accumulate on top.
                    nc.scalar.copy(deng[g][:], d2h)
                    nc.vector.memset(sampg[g][:], 0.0)

                def emit_score_pair(cp):
                    sc = pA.tile([128, 1024], F32, tag="pa", name=f"sc{cp}")
                    spans = []
                    for ci in range(2):
                        c = 2 * cp + ci
                        lo, hi = max(0, c - 2), min(NSUB - 1, c)
                        alo = lo - (c - 2)
                        spans.append((alo, alo + hi - lo + 1))
                        nc.tensor.matmul(
                            sc[:, 512 * ci + 128 * alo:512 * ci + 128 * (alo + hi - lo + 1)],
                            hpad[:, 128 * c:128 * (c + 1)],
                            hpad[:, 128 * (lo + 1):128 * (hi + 2)],
                            start=True, stop=True)
                    meng = nc.vector
                    if spans == [(0, 3), (0, 3)]:
                        asl = attnm[:, 1024 * cp:1024 * cp + 896]
                        nc.scalar.activation(asl, sc[:, 0:896], AF.Exp,
                                             scale=float(SCALE))
                        meng.tensor_tensor(out=asl, in0=asl,
                                           in1=maskC2g, op=ALU.mult)
                    else:
                        for ci, (a0, a1) in enumerate(spans):
                            ss = slice(512 * ci + 128 * a0, 512 * ci + 128 * a1)
                            asl = attnm[:, 1024 * cp + ss.start:1024 * cp + ss.stop]
                            nc.scalar.activation(asl, sc[:, ss], AF.Exp,
                                                 scale=float(SCALE))
                            nc.vector.tensor_tensor(
                                out=asl, in0=asl,
                                in1=maskC[:, 128 * a0:128 * a1], op=ALU.mult)

                def emit_densamp_chunk(c):
                    parts = _chunk_parts(c)
                    for g, s, e in parts:
                        aa = s - (c - 2)
                        rhs = attnm[:, 512 * c + 128 * aa:512 * c + 128 * (aa + e - s + 1)]
                        nc.tensor.matmul(
                            deng[g][:, 128 * (s - 4 * g):128 * (e + 1 - 4 * g)],
                            onesb, rhs, start=False, stop=False,
                            skip_group_check=True)
                    for g, s, e in parts:
                        aa = s - (c - 2)
                        nc.tensor.matmul(
                            sampg[g][:, 128 * (s - 4 * g):128 * (e + 1 - 4 * g)],
                            hT[:, 128 * c:128 * (c + 1)],
                            attnm[:, 512 * c + 128 * aa:512 * c + 128 * (aa + e - s + 1)],
                            start=False, stop=False, skip_group_check=True)

                def emit_finish_a(g, q0, q1):
                    # normalize subrange [128*q0, 128*q1) of group g's bank
                    gsl = slice(512 * g + 128 * q0, 512 * g + 128 * q1)
                    bsl = slice(128 * q0, 128 * q1)
                    # z = 1 / (den + D): den banks were preset with D, so a
                    # plain full-width reciprocal does it (den is broadcast
                    # across partitions by the ones stationary).  The
                    # softmax 1/2 vs gate 2x cancels via Wout/2 on host.
                    z = smp.tile([128, 512], F32, tag="z", name=f"z{g}_{q0}")
                    zc = z[:, 0:128 * (q1 - q0)]
                    nc.vector.reciprocal_approx_fast(zc, deng[g][:, bsl])
                    # attr = (samp * Pg) * z
                    nc.vector.tensor_tensor(out=attr[:, gsl], in0=sampg[g][:, bsl],
                                            in1=Pg[:, gsl], op=ALU.mult)
                    nc.vector.tensor_tensor(out=attr[:, gsl], in0=attr[:, gsl],
                                            in1=zc, op=ALU.mult)

                def emit_finish_b(g, q0, q1):
                    # output conv + bias + store, one ds-pair after finish_a
                    # so the PE never waits on the DVE normalize chain
                    gsl = slice(512 * g + 128 * q0, 512 * g + 128 * q1)
                    w = 128 * (q1 - q0)
                    po = pA.tile([128, 1024], F32, tag="pa", name=f"po{g}_{q0}")
                    nc.tensor.matmul(po[:, 0:w], wot[:, 0:128], attr[:, gsl],
                                     start=True, stop=True)
                    nc.tensor.matmul(po[:, 512:512 + w], wot[:, 128:256],
                                     attr[:, gsl], start=True, stop=True)
                    ot = otp.tile([128, 1024], BF16, tag="ot")
                    nc.scalar.activation(ot[:, 0:w], po[:, 0:w],
                                         AF.Identity, bias=bout0, scale=1.0)
                    nc.scalar.activation(ot[:, 512:512 + w], po[:, 512:512 + w],
                                         AF.Identity, bias=bout1, scale=1.0)
                    for oc in range(2):
                        nc.sync.dma_start(out_d[128 * oc:128 * (oc + 1), gsl],
                                          ot[:, 512 * oc:512 * oc + w])

                fb_queue = []

                def emit_dsp(dp):
                    while fb_queue and fb_queue[0][3] < dp:
                        g, q0, q1, _ = fb_queue.pop(0)
                        emit_finish_b(g, q0, q1)
                    gmax = min(NSUB - 1, 2 * dp + 3) // 4
                    for g in range(gmax + 1):
                        ensure_group(g)
                    for c in (2 * dp, 2 * dp + 1):
                        emit_densamp_chunk(c)
                        if c >= 5 and (c - 5) % 4 == 0 and c < 33:
                            g = (c - 5) // 4
                            emit_finish_a(g, 0, 4)
                            fb_queue.append((g, 0, 4, dp))
                        elif c == 31:
                            # last group in two halves to shorten the tail:
                            # subs 28,29 close at chunk 31...
                            emit_finish_a(NGRP - 1, 0, 2)
                            fb_queue.append((NGRP - 1, 0, 2, dp))
                        elif c == 33:
                            # ...subs 30,31 at chunk 33
                            emit_finish_a(NGRP - 1, 2, 4)
                            fb_queue.append((NGRP - 1, 2, 4, dp))

                # ---- interleaved schedule: conv1/transpose/conv2 (gated on
                # the x stream) with the score pipeline filling DMA-wait
                # gaps, then the chunk pipeline with den/samp trailing the
                # scores and group outputs streaming as denominators close.
                emit_conv_u(0)
                emit_conv_u(1)
                emit_transp_group(0)
                emit_conv2(0)
                emit_conv2(1)
                emit_score_pair(0)
                emit_score_pair(1)
                emit_score_pair(2)
                ensure_group(0)
                ensure_group(1)
                emit_conv_u(2)
                emit_transp_group(1)
                emit_conv2(2)
                emit_conv2(3)
                emit_score_pair(3)
                emit_score_pair(4)
                emit_dsp(0)
                emit_score_pair(5)
                emit_score_pair(6)
                emit_dsp(1)
                emit_conv_u(3)
                emit_transp_group(2)
                emit_conv2(4)
                emit_conv2(5)
                emit_score_pair(7)
                emit_dsp(2)
                emit_score_pair(8)
                emit_dsp(3)
                emit_transp_group(3)
                emit_conv2(6)
                emit_conv2(7)
                dp = 4
                for cp in range(9, 21):
                    if cp <= 16:
                        emit_score_pair(cp)
                    nds = 2 if cp == 9 else 1
                    for _ in range(nds):
                        if dp <= 16:
                            emit_dsp(dp)
                            dp += 1
                while fb_queue:
                    g, q0, q1, _ = fb_queue.pop(0)
                    emit_finish_b(g, q0, q1)

    return nc


def _prep_inputs(x, W1, b1, W2, b2, Wout, bout):
    m2g, d2h = _build_masks()
    bf = ml_dtypes.bfloat16

    blob_bf = np.zeros((128, BLOBW), dtype=bf)

    def put_bf(col, arr):
        arr = np.asarray(arr).astype(bf)
        blob_bf[:arr.shape[0], col:col + arr.shape[1]] = arr

    def put_f32(col, arr):
        arr = np.ascontiguousarray(np.asarray(arr, np.float32))
        v = arr.view(np.uint16).reshape(arr.shape[0], -1)
        blob_bf[:arr.shape[0], col:col + v.shape[1]] = v.view(bf)

    put_bf(O_W2T, np.ascontiguousarray(W2.T))
    # Wout/2 absorbs the softmax 1/2 left over from the 2*sigmoid gate
    put_bf(O_WOT, np.ascontiguousarray(np.asarray(Wout, np.float32).T * 0.5))
    put_bf(O_M2G, m2g)
    put_bf(O_IDENT, np.eye(128, dtype=np.float32))
    put_bf(O_ONESB, np.ones((128, 128), np.float32))
    put_f32(O_B1, np.asarray(b1, np.float32).reshape(CH, 1))
    put_f32(O_B2H, (0.5 * np.asarray(b2, np.float32)).reshape(CH, 1))
    put_f32(O_BOUT, np.ascontiguousarray(
        np.asarray(bout, np.float32).reshape(2, CH).T))
    put_bf(O_D2H, d2h)

    common = {
        "W1T": np.ascontiguousarray(W1.T).astype(bf),
        "blob": blob_bf,
    }
    in_maps = []
    for i in range(N):
        m = dict(common)
        m["x"] = np.ascontiguousarray(
            np.asarray(x[i], np.float32).reshape(CIN, HW)).astype(bf)
        in_maps.append(m)
    return in_maps


_CACHED = {}


def kernel(x, W1, b1, W2, b2, Wout, bout):
    if "nc" not in _CACHED:
        nc = build_nc()
        nc.finalize()
        _CACHED["nc"] = nc
    nc = _CACHED["nc"]
    in_maps = _prep_inputs(x, W1, b1, W2, b2, Wout, bout)
    res = run_bass_kernel_spmd(nc, in_maps, core_ids=list(range(N)))
    out = np.stack([np.asarray(res.results[i]["out"], dtype=np.float32)
                    .reshape(CIN, H, W) for i in range(N)])
    return out


# revision 39
# speedup vs baseline: 1.3631x; 1.0180x over previous
"""Trainium2 Bass kernel for nn_Attribution (sparse local-window attention).

Data-parallel over batch n=8 -> one batch element per NeuronCore.

Per-core computation (c_in=256, ch=128, 64x64 image):
    h    = W1 @ x + b1
    corr = 5x5 local window correlation of h (zero padded), /sqrt(128)
    attn = softmax over the 25 window entries
    samp = sum_k attn_k * shift_k(h)
    gate = sigmoid(relu(W2 @ h + b2)) = 0.5 + 0.5*relu(tanh((z+b2)/2))
    out  = Wout @ (gate * samp) + bout

Layout: positions flattened row-major with 2 zero-pad rows top/bottom
(68 rows x 64 = 4352 positions = 34 chunks of 128).  Scores "born
transposed" (keys of chunk c on partitions, queries on free axis).
Out-of-window entries killed by a {0,1} mask after exp; out-of-image x
neighbors accounted by denominator correction D (exp(0)=1 each in the
zero-padded reference).

This version keeps the PE stream minimal and the output phase fused into
the chunk pipeline:
  - den/samp PSUM banks are pre-zeroed by memsets on DVE/GPSIMD (idle
    engines), so every den/samp matmul is a plain accumulate and the 16
    pre-zero PE matmuls of the previous version are gone,
  - den uses an all-ones [128,128] stationary so the column sums land
    broadcast across all partitions: the reciprocal is computed full-width
    on DVE and the PE partition-broadcast matmuls are gone,
  - each group's normalize + output conv + bias + store is emitted as soon
    as its denominator closes, so output DMA streams during the chunk
    pipeline instead of draining at the end,
  - evacuations are spread: ACT does only exp/tanh, DVE does mask/recip/
    normalize/conv1-bias, GPSIMD does transpose-evac/gate/attr/out-bias,
  - input rides in 8 fat x DMAs + 2 weight + 2 blob DMAs over 4 queues.
"""
import sys

sys.path.insert(0, "/opt/trn_rl_repo")

import numpy as np
import ml_dtypes

import concourse.bass as bass
import concourse.mybir as mybir
import concourse.tile as tile
from concourse import bacc
from concourse.bass_utils import run_bass_kernel_spmd

F32 = mybir.dt.float32
BF16 = mybir.dt.bfloat16
AF = mybir.ActivationFunctionType
ALU = mybir.AluOpType

N, CIN, CH, H, W = 8, 256, 128, 64, 64
HW = H * W                      # 4096
RAD = 2
KROWS = H + 2 * RAD             # 68 padded rows
PADPOS = KROWS * W              # 4352
NCHUNK = PADPOS // 128          # 34 key chunks (2 rows each)
NSUB = H // 2                   # 32 query subs (128 queries each)
NGRP = NSUB // 4                # 8 groups of 4 subs (one PSUM bank each)
SCALE = 1.0 / np.sqrt(np.float32(CH))

# ---- const blob layout (bf16 [128, BLOBW]) ----
O_W2T = 0            # [128,128]
O_WOT = 128          # [128,256]
O_M2G = 384          # maskC2g [128,896]
O_IDENT = 1280       # [128,128]
O_ONESB = 1408       # [128,128] all-ones
O_B1 = 1536          # [128,1] f32 (2 bf16 cols)
O_B2H = 1538         # [128,1] f32
O_BOUT = 1540        # [128,2] f32 (4 bf16 cols)
O_D2C = 1544         # [128,512] bf16: D(q)/128 tiled, identical rows
BLOBW = 2056


def _build_masks():
    """maskC2g: (128, 896) {0,1} bf16 = maskC | zeros(128) | maskC.
    maskC col 128*a+q is key (chunk c, pos p) vs query q of sub s=c-2+a:
    valid iff |2-2a + p//64 - q//64| <= 2 and |p%64 - q%64| <= 2."""
    m = np.zeros((128, 384), dtype=np.float32)
    for a in range(3):
        for p in range(128):
            for q in range(128):
                dy = 2 - 2 * a + p // 64 - q // 64
                if abs(dy) <= RAD and abs(p % 64 - q % 64) <= RAD:
                    m[p, 128 * a + q] = 1.0
    m2g = np.concatenate([m, np.zeros((128, 128), np.float32), m], axis=1)

    cnt = np.array([sum(1 for dx in range(-RAD, RAD + 1) if not 0 <= qx + dx < W)
                    for qx in range(W)], dtype=np.float32)
    drow = (5.0 / 128.0) * np.concatenate([cnt, cnt])       # (128,) D/128
    d2c = np.tile(np.tile(drow, 4)[None, :], (128, 1))      # (128,512)
    return m2g.astype(ml_dtypes.bfloat16), d2c.astype(ml_dtypes.bfloat16)


def _chunk_parts(c):
    """den/samp MM parts for chunk c: (g, s_lo, s_hi) sub-ranges split at
    4-sub PSUM bank boundaries.  Banks are pre-zeroed by memsets, so every
    part is a plain accumulate."""
    smin, smax = max(0, c - 2), min(NSUB - 1, c)
    parts = []
    for g in range(smin // 4, smax // 4 + 1):
        parts.append((g, max(smin, 4 * g), min(smax, 4 * g + 3)))
    return parts


def build_nc(repeat=1, sim_safe=False):
    nc = bacc.Bacc("TRN2", target_bir_lowering=False, debug=False, num_devices=8)

    x_d = nc.declare_dram_parameter("x", [CIN, HW], BF16, isOutput=False)
    w1t_d = nc.declare_dram_parameter("W1T", [CIN, CH], BF16, isOutput=False)
    blob_d = nc.declare_dram_parameter("blob", [128, BLOBW], BF16, isOutput=False)
    out_d = nc.declare_dram_parameter("out", [CIN, HW], BF16, isOutput=True)

    with tile.TileContext(nc) as tc:
        with (
            tc.tile_pool(name="per", bufs=1) as per,
            tc.tile_pool(name="smp", bufs=8) as smp,
            tc.tile_pool(name="otp", bufs=4) as otp,
            tc.tile_pool(name="pA", bufs=2, space="PSUM") as pA,   # 2x[128,1024] f32
            tc.tile_pool(name="pB", bufs=2, space="PSUM") as pB,   # 2x[128,512] f32 samp
            tc.tile_pool(name="pD", bufs=2, space="PSUM") as pD,   # 2x[128,512] f32 den
        ):
            blobw = per.tile([128, 2 * CH], BF16, tag="blobw")
            blob = per.tile([128, BLOBW], BF16, tag="blob")
            xall = per.tile([128, 2 * HW], BF16, tag="xall")
            hpad = per.tile([128, PADPOS], BF16, tag="hpad")
            hT = per.tile([128, PADPOS], BF16, tag="hT")
            attnm = per.tile([128, NCHUNK * 512], BF16, tag="attnm")
            Pg = per.tile([128, HW], BF16, tag="Pg")
            attr = per.tile([128, HW], BF16, tag="attr")

            w1t0 = blobw[:, 0:CH]
            w1t1 = blobw[:, CH:2 * CH]
            w2t = blob[:, O_W2T:O_W2T + 128]
            wot = blob[:, O_WOT:O_WOT + 256]
            maskC2g = blob[:, O_M2G:O_M2G + 896]
            maskC = blob[:, O_M2G:O_M2G + 384]
            ident = blob[:, O_IDENT:O_IDENT + 128]
            onesb = blob[:, O_ONESB:O_ONESB + 128]
            b1 = blob[:, O_B1:O_B1 + 2].bitcast(F32)
            b2h = blob[:, O_B2H:O_B2H + 2].bitcast(F32)
            bout0 = blob[:, O_BOUT:O_BOUT + 2].bitcast(F32)
            bout1 = blob[:, O_BOUT + 2:O_BOUT + 4].bitcast(F32)
            d2c = blob[:, O_D2C:O_D2C + 512]

            # --- input DMAs over 4 issue queues.  Per queue: the weight /
            # blob piece that queue owns, then x blocks in consumption
            # order.  Each dma_start is packetized across all 16 HW DMA
            # engines, so few fat transfers saturate the ~250GB/s link.
            def xdma(eng, half, u):
                src = x_d[128 * half:128 * (half + 1), 1024 * u:1024 * (u + 1)]
                eng.dma_start(
                    xall[:, HW * half + 1024 * u:HW * half + 1024 * (u + 1)], src)

            nc.sync.dma_start(blobw[:, 0:CH], w1t_d[0:128, :])
            nc.scalar.dma_start(blobw[:, CH:2 * CH], w1t_d[128:256, :])
            # first blocks split fine so the first conv matmuls start early
            nc.sync.dma_start(xall[:, 0:256], x_d[0:128, 0:256])
            nc.scalar.dma_start(xall[:, HW:HW + 256], x_d[128:256, 0:256])
            # small early piece with ident/onesb/biases: nothing downstream
            # ever waits on the fat blob transfers for these
            nc.gpsimd.dma_start(blob[:, O_IDENT:O_D2C], blob_d[:, O_IDENT:O_D2C])
            nc.sync.dma_start(xall[:, 256:512], x_d[0:128, 256:512])
            nc.scalar.dma_start(xall[:, HW + 256:HW + 512], x_d[128:256, 256:512])
            nc.gpsimd.dma_start(blob[:, O_D2C:BLOBW], blob_d[:, O_D2C:BLOBW])
            nc.sync.dma_start(xall[:, 512:1024], x_d[0:128, 512:1024])
            nc.scalar.dma_start(xall[:, HW + 512:HW + 1024], x_d[128:256, 512:1024])
            xdma(nc.gpsimd, 0, 1)
            xdma(nc.sync, 1, 1)
            nc.scalar.dma_start(blob[:, 0:O_IDENT], blob_d[:, 0:O_IDENT])
            xdma(nc.gpsimd, 0, 2)
            xdma(nc.sync, 1, 2)
            xdma(nc.scalar, 0, 3)
            xdma(nc.gpsimd, 1, 3)

            # pad chunks (0 and 33) are identically zero
            nc.vector.memset(hpad[:, 0:128], 0.0)
            nc.vector.memset(hpad[:, PADPOS - 128:PADPOS], 0.0)
            nc.gpsimd.memset(hT[:, 0:128], 0.0)
            nc.gpsimd.memset(hT[:, PADPOS - 128:PADPOS], 0.0)



            for _rep in range(repeat):
                # ---- P1: conv1 + transposes + conv2, PE kept streaming.
                def emit_transp_group(u):
                    pt = pA.tile([128, 1024], BF16, tag="pa", name=f"pt{u}")
                    for k in range(8):
                        c = 8 * u + 1 + k
                        nc.tensor.transpose(pt[:, 128 * k:128 * (k + 1)],
                                            hpad[:, 128 * c:128 * (c + 1)],
                                            ident)
                    nc.scalar.copy(hT[:, 128 * (8 * u + 1):128 * (8 * u + 9)],
                                   pt[:])

                def emit_conv2(b):
                    pz = pB.tile([128, 512], F32, tag="pb", name=f"pz{b}")
                    nc.tensor.matmul(pz[:], w2t,
                                     hpad[:, 128 + 512 * b:128 + 512 * (b + 1)],
                                     start=True, stop=True)
                    tg = smp.tile([128, 512], BF16, tag="tg")
                    nc.scalar.activation(tg[:], pz[:], AF.Tanh, scale=0.5, bias=b2h)
                    nc.vector.tensor_scalar(
                        out=Pg[:, 512 * b:512 * (b + 1)], in0=tg[:],
                        scalar1=0.0, scalar2=1.0, op0=ALU.max, op1=ALU.add)

                def emit_conv_u(u):
                    cvt = pA.tile([128, 1024], F32, tag="pa", name=f"cv{u}")
                    for h2 in range(2):
                        # first block in 256-col pieces: starts as soon as
                        # the first fine x DMAs land
                        npc = 2 if u == 0 and h2 == 0 else 1
                        mms = []
                        for j in range(npc):
                            w = 512 // npc
                            dst = cvt[:, 512 * h2 + w * j:512 * h2 + w * (j + 1)]
                            cs = slice(1024 * u + 512 * h2 + w * j,
                                       1024 * u + 512 * h2 + w * (j + 1))
                            cs2 = slice(HW + cs.start, HW + cs.stop)
                            mms.append((dst, w1t0, xall[:, cs]))
                            mms.append((dst, w1t1, xall[:, cs2]))
                        for k, (dst, lh, rh) in enumerate(mms):
                            nc.tensor.matmul(dst, lh, rh, start=k == 0,
                                             stop=k == len(mms) - 1)
                    nc.vector.tensor_scalar(
                        out=hpad[:, 128 + 1024 * u:128 + 1024 * (u + 1)],
                        in0=cvt[:], scalar1=b1, scalar2=None, op0=ALU.add)

                deng = {}
                sampg = {}

                def ensure_group(g):
                    if g in deng or g >= NGRP:
                        return
                    deng[g] = pD.tile([128, 512], F32, tag="pd", name=f"dn{g}")
                    sampg[g] = pB.tile([128, 512], F32, tag="pb", name=f"sp{g}")
                    # den preset = D(q) via a full-width PE matmul (colsum of
                    # d2c is exactly D): also scrubs any stale pending-zero
                    # state in the bank.  samp = 0 via DVE memset (its bank
                    # was start=True full-written by a conv2 tile earlier).
                    nc.tensor.matmul(deng[g][:], onesb, d2c, start=True, stop=True)
                    nc.vector.memset(sampg[g][:], 0.0)

                def emit_score_pair(cp):
                    sc = pA.tile([128, 1024], F32, tag="pa", name=f"sc{cp}")
                    spans = []
                    for ci in range(2):
                        c = 2 * cp + ci
                        lo, hi = max(0, c - 2), min(NSUB - 1, c)
                        alo = lo - (c - 2)
                        spans.append((alo, alo + hi - lo + 1))
                        nc.tensor.matmul(
                            sc[:, 512 * ci + 128 * alo:512 * ci + 128 * (alo + hi - lo + 1)],
                            hpad[:, 128 * c:128 * (c + 1)],
                            hpad[:, 128 * (lo + 1):128 * (hi + 2)],
                            start=True, stop=True)
                    if spans == [(0, 3), (0, 3)]:
                        # one exp + one mask over both 384-wide chunk blocks,
                        # skipping the 128-col gap between them (3D APs)
                        asl = attnm[:, 1024 * cp:1024 * (cp + 1)].rearrange(
                            "p (c w) -> p c w", c=2)[:, :, 0:384]
                        sc3 = sc[:].rearrange("p (c w) -> p c w", c=2)[:, :, 0:384]
                        nc.scalar.activation(asl, sc3, AF.Exp,
                                             scale=float(SCALE))
                        nc.vector.tensor_tensor(
                            out=asl, in0=asl,
                            in1=maskC.unsqueeze(1).broadcast_to([128, 2, 384]),
                            op=ALU.mult)
                    else:
                        for ci, (a0, a1) in enumerate(spans):
                            ss = slice(512 * ci + 128 * a0, 512 * ci + 128 * a1)
                            asl = attnm[:, 1024 * cp + ss.start:1024 * cp + ss.stop]
                            nc.scalar.activation(asl, sc[:, ss], AF.Exp,
                                                 scale=float(SCALE))
                            nc.vector.tensor_tensor(
                                out=asl, in0=asl,
                                in1=maskC[:, 128 * a0:128 * a1], op=ALU.mult)

                def emit_densamp_chunk(c):
                    parts = _chunk_parts(c)
                    for g, s, e in parts:
                        aa = s - (c - 2)
                        rhs = attnm[:, 512 * c + 128 * aa:512 * c + 128 * (aa + e - s + 1)]
                        nc.tensor.matmul(
                            deng[g][:, 128 * (s - 4 * g):128 * (e + 1 - 4 * g)],
                            onesb, rhs, start=False, stop=False,
                            skip_group_check=True)
                    for g, s, e in parts:
                        aa = s - (c - 2)
                        nc.tensor.matmul(
                            sampg[g][:, 128 * (s - 4 * g):128 * (e + 1 - 4 * g)],
                            hT[:, 128 * c:128 * (c + 1)],
                            attnm[:, 512 * c + 128 * aa:512 * c + 128 * (aa + e - s + 1)],
                            start=False, stop=False, skip_group_check=True)

                def emit_finish_a(g, q0, q1):
                    # normalize subrange [128*q0, 128*q1) of group g's bank
                    gsl = slice(512 * g + 128 * q0, 512 * g + 128 * q1)
                    bsl = slice(128 * q0, 128 * q1)
                    # z = 1 / (den + D): den banks were preset with D, so a
                    # plain full-width reciprocal does it (den is broadcast
                    # across partitions by the ones stationary).  The
                    # softmax 1/2 vs gate 2x cancels via Wout/2 on host.
                    z = smp.tile([128, 512], F32, tag="z", name=f"z{g}_{q0}")
                    zc = z[:, 0:128 * (q1 - q0)]
                    nc.vector.reciprocal_approx_fast(zc, deng[g][:, bsl])
                    # attr = (samp * Pg) * z
                    nc.vector.tensor_tensor(out=attr[:, gsl], in0=sampg[g][:, bsl],
                                            in1=Pg[:, gsl], op=ALU.mult)
                    nc.vector.tensor_tensor(out=attr[:, gsl], in0=attr[:, gsl],
                                            in1=zc, op=ALU.mult)

                def emit_finish_b(g, q0, q1):
                    # output conv + bias + store, one ds-pair after finish_a
                    # so the PE never waits on the DVE normalize chain
                    gsl = slice(512 * g + 128 * q0, 512 * g + 128 * q1)
                    w = 128 * (q1 - q0)
                    po = pA.tile([128, 1024], F32, tag="pa", name=f"po{g}_{q0}")
                    nc.tensor.matmul(po[:, 0:w], wot[:, 0:128], attr[:, gsl],
                                     start=True, stop=True)
                    nc.tensor.matmul(po[:, 512:512 + w], wot[:, 128:256],
                                     attr[:, gsl], start=True, stop=True)
                    ot = otp.tile([128, 1024], BF16, tag="ot")
                    nc.scalar.activation(ot[:, 0:w], po[:, 0:w],
                                         AF.Identity, bias=bout0, scale=1.0)
                    nc.scalar.activation(ot[:, 512:512 + w], po[:, 512:512 + w],
                                         AF.Identity, bias=bout1, scale=1.0)
                    for oc in range(2):
                        nc.sync.dma_start(out_d[128 * oc:128 * (oc + 1), gsl],
                                          ot[:, 512 * oc:512 * oc + w])

                fb_queue = []

                def emit_dsp(dp):
                    while fb_queue and fb_queue[0][3] < dp:
                        g, q0, q1, _ = fb_queue.pop(0)
                        emit_finish_b(g, q0, q1)
                    gmax = min(NSUB - 1, 2 * dp + 3) // 4
                    for g in range(gmax + 1):
                        ensure_group(g)
                    for c in (2 * dp, 2 * dp + 1):
                        emit_densamp_chunk(c)
                        if c >= 5 and (c - 5) % 4 == 0 and c < 33:
                            g = (c - 5) // 4
                            emit_finish_a(g, 0, 4)
                            fb_queue.append((g, 0, 4, dp))
                        elif c == 31:
                            # last group in two halves to shorten the tail:
                            # subs 28,29 close at chunk 31...
                            emit_finish_a(NGRP - 1, 0, 2)
                            fb_queue.append((NGRP - 1, 0, 2, dp))
                        elif c == 33:
                            # ...subs 30,31 at chunk 33
                            emit_finish_a(NGRP - 1, 2, 4)
                            fb_queue.append((NGRP - 1, 2, 4, dp))

                # ---- interleaved schedule: conv1/transpose/conv2 (gated on
                # the x stream) with the score pipeline filling DMA-wait
                # gaps, then the chunk pipeline with den/samp trailing the
                # scores and group outputs streaming as denominators close.
                emit_conv_u(0)
                emit_conv_u(1)
                emit_transp_group(0)
                emit_conv2(0)
                emit_conv2(1)
                emit_score_pair(0)
                emit_score_pair(1)
                emit_score_pair(2)
                ensure_group(0)
                ensure_group(1)
                emit_conv_u(2)
                emit_transp_group(1)
                emit_conv2(2)
                emit_conv2(3)
                emit_score_pair(3)
                emit_score_pair(4)
                emit_dsp(0)
                emit_score_pair(5)
                emit_score_pair(6)
                emit_dsp(1)
                emit_conv_u(3)
                emit_transp_group(2)
                emit_conv2(4)
                emit_conv2(5)
                emit_score_pair(7)
                emit_dsp(2)
                emit_score_pair(8)
                emit_dsp(3)
                emit_transp_group(3)
                emit_conv2(6)
                emit_conv2(7)
                dp = 4
                for cp in range(9, 21):
                    if cp <= 16:
                        emit_score_pair(cp)
                    nds = 2 if cp == 9 else 1
                    for _ in range(nds):
                        if dp <= 16:
                            emit_dsp(dp)
                            dp += 1
                while fb_queue:
                    g, q0, q1, _ = fb_queue.pop(0)
                    emit_finish_b(g, q0, q1)

    return nc


def _prep_inputs(x, W1, b1, W2, b2, Wout, bout):
    m2g, d2c = _build_masks()
    bf = ml_dtypes.bfloat16

    blob_bf = np.zeros((128, BLOBW), dtype=bf)

    def put_bf(col, arr):
        arr = np.asarray(arr).astype(bf)
        blob_bf[:arr.shape[0], col:col + arr.shape[1]] = arr

    def put_f32(col, arr):
        arr = np.ascontiguousarray(np.asarray(arr, np.float32))
        v = arr.view(np.uint16).reshape(arr.shape[0], -1)
        blob_bf[:arr.shape[0], col:col + v.shape[1]] = v.view(bf)

    put_bf(O_W2T, np.ascontiguousarray(W2.T))
    # Wout/2 absorbs the softmax 1/2 left over from the 2*sigmoid gate
    put_bf(O_WOT, np.ascontiguousarray(np.asarray(Wout, np.float32).T * 0.5))
    put_bf(O_M2G, m2g)
    put_bf(O_IDENT, np.eye(128, dtype=np.float32))
    put_bf(O_ONESB, np.ones((128, 128), np.float32))
    put_f32(O_B1, np.asarray(b1, np.float32).reshape(CH, 1))
    put_f32(O_B2H, (0.5 * np.asarray(b2, np.float32)).reshape(CH, 1))
    put_f32(O_BOUT, np.ascontiguousarray(
        np.asarray(bout, np.float32).reshape(2, CH).T))
    put_bf(O_D2C, d2c)

    common = {
        "W1T": np.ascontiguousarray(W1.T).astype(bf),
        "blob": blob_bf,
    }
    in_maps = []
    for i in range(N):
        m = dict(common)
        m["x"] = np.ascontiguousarray(
            np.asarray(x[i], np.float32).reshape(CIN, HW)).astype(bf)
        in_maps.append(m)
    return in_maps


_CACHED = {}


def kernel(x, W1, b1, W2, b2, Wout, bout):
    if "nc" not in _CACHED:
        nc = build_nc()
        nc.finalize()
        _CACHED["nc"] = nc
    nc = _CACHED["nc"]
    in_maps = _prep_inputs(x, W1, b1, W2, b2, Wout, bout)
    res = run_bass_kernel_spmd(nc, in_maps, core_ids=list(range(N)))
    out = np.stack([np.asarray(res.results[i]["out"], dtype=np.float32)
                    .reshape(CIN, H, W) for i in range(N)])
    return out


# revision 42
# speedup vs baseline: 1.4324x; 1.0508x over previous
"""Trainium2 Bass kernel for nn_Attribution (sparse local-window attention).

Data-parallel over batch n=8 -> one batch element per NeuronCore.

Per-core computation (c_in=256, ch=128, 64x64 image):
    h    = W1 @ x + b1
    corr = 5x5 local window correlation of h (zero padded), /sqrt(128)
    attn = softmax over the 25 window entries
    samp = sum_k attn_k * shift_k(h)
    gate = sigmoid(relu(W2 @ h + b2)) = 0.5 + 0.5*relu(tanh((z+b2)/2))
    out  = Wout @ (gate * samp) + bout

Layout: positions flattened row-major with 2 zero-pad rows top/bottom
(68 rows x 64 = 4352 positions = 34 chunks of 128).  Scores "born
transposed" (keys of chunk c on partitions, queries on free axis).
Out-of-window entries killed by a {0,1} mask after exp; out-of-image x
neighbors accounted by denominator correction D (exp(0)=1 each in the
zero-padded reference).

This version keeps the PE stream minimal and the output phase fused into
the chunk pipeline:
  - den/samp PSUM banks are pre-zeroed by memsets on DVE/GPSIMD (idle
    engines), so every den/samp matmul is a plain accumulate and the 16
    pre-zero PE matmuls of the previous version are gone,
  - den uses an all-ones [128,128] stationary so the column sums land
    broadcast across all partitions: the reciprocal is computed full-width
    on DVE and the PE partition-broadcast matmuls are gone,
  - each group's normalize + output conv + bias + store is emitted as soon
    as its denominator closes, so output DMA streams during the chunk
    pipeline instead of draining at the end,
  - evacuations are spread: ACT does only exp/tanh, DVE does mask/recip/
    normalize/conv1-bias, GPSIMD does transpose-evac/gate/attr/out-bias,
  - input rides in 8 fat x DMAs + 2 weight + 2 blob DMAs over 4 queues.
"""
import sys

sys.path.insert(0, "/opt/trn_rl_repo")

import numpy as np
import ml_dtypes

import concourse.bass as bass
import concourse.mybir as mybir
import concourse.tile as tile
from concourse import bacc
from concourse.bass_utils import run_bass_kernel_spmd

F32 = mybir.dt.float32
BF16 = mybir.dt.bfloat16
AF = mybir.ActivationFunctionType
ALU = mybir.AluOpType

N, CIN, CH, H, W = 8, 256, 128, 64, 64
HW = H * W                      # 4096
RAD = 2
KROWS = H + 2 * RAD             # 68 padded rows
PADPOS = KROWS * W              # 4352
NCHUNK = PADPOS // 128          # 34 key chunks (2 rows each)
NSUB = H // 2                   # 32 query subs (128 queries each)
NGRP = NSUB // 4                # 8 groups of 4 subs (one PSUM bank each)
SCALE = 1.0 / np.sqrt(np.float32(CH))

# ---- const blob layout (bf16 [128, BLOBW]) ----
O_W2T = 0            # [128,128]
O_WOT = 128          # [128,256]
O_M2G = 384          # maskC2g [128,896]
O_IDENT = 1280       # [128,128]
O_ONESB = 1408       # [128,128] all-ones
O_B1 = 1536          # [128,1] f32 (2 bf16 cols)
O_B2H = 1538         # [128,1] f32
O_BOUT = 1540        # [128,2] f32 (4 bf16 cols)
O_D2C = 1544         # [128,512] bf16: D(q)/128 tiled, identical rows
BLOBW = 2056


def _build_masks():
    """maskC2g: (128, 896) {0,1} bf16 = maskC | zeros(128) | maskC.
    maskC col 128*a+q is key (chunk c, pos p) vs query q of sub s=c-2+a:
    valid iff |2-2a + p//64 - q//64| <= 2 and |p%64 - q%64| <= 2."""
    m = np.zeros((128, 384), dtype=np.float32)
    for a in range(3):
        for p in range(128):
            for q in range(128):
                dy = 2 - 2 * a + p // 64 - q // 64
                if abs(dy) <= RAD and abs(p % 64 - q % 64) <= RAD:
                    m[p, 128 * a + q] = 1.0
    m2g = np.concatenate([m, np.zeros((128, 128), np.float32), m], axis=1)

    cnt = np.array([sum(1 for dx in range(-RAD, RAD + 1) if not 0 <= qx + dx < W)
                    for qx in range(W)], dtype=np.float32)
    drow = (5.0 / 128.0) * np.concatenate([cnt, cnt])       # (128,) D/128
    d2c = np.tile(np.tile(drow, 4)[None, :], (128, 1))      # (128,512)
    return m2g.astype(ml_dtypes.bfloat16), d2c.astype(ml_dtypes.bfloat16)


def _chunk_parts(c):
    """den/samp MM parts for chunk c: (g, s_lo, s_hi) sub-ranges split at
    4-sub PSUM bank boundaries.  Banks are pre-zeroed by memsets, so every
    part is a plain accumulate."""
    smin, smax = max(0, c - 2), min(NSUB - 1, c)
    parts = []
    for g in range(smin // 4, smax // 4 + 1):
        parts.append((g, max(smin, 4 * g), min(smax, 4 * g + 3)))
    return parts


def build_nc(repeat=1, sim_safe=False):
    nc = bacc.Bacc("TRN2", target_bir_lowering=False, debug=False, num_devices=8)

    x_d = nc.declare_dram_parameter("x", [CIN, HW], BF16, isOutput=False)
    w1t_d = nc.declare_dram_parameter("W1T", [CIN, CH], BF16, isOutput=False)
    blob_d = nc.declare_dram_parameter("blob", [128, BLOBW], BF16, isOutput=False)
    out_d = nc.declare_dram_parameter("out", [CIN, HW], BF16, isOutput=True)

    with tile.TileContext(nc) as tc:
        with (
            tc.tile_pool(name="per", bufs=1) as per,
            tc.tile_pool(name="smp", bufs=8) as smp,
            tc.tile_pool(name="otp", bufs=4) as otp,
            tc.tile_pool(name="pA", bufs=2, space="PSUM") as pA,   # 2x[128,1024] f32
            tc.tile_pool(name="pB", bufs=2, space="PSUM") as pB,   # 2x[128,512] f32 samp
            tc.tile_pool(name="pD", bufs=2, space="PSUM") as pD,   # 2x[128,512] f32 den
        ):
            blobw = per.tile([128, 2 * CH], BF16, tag="blobw")
            blob = per.tile([128, BLOBW], BF16, tag="blob")
            xall = per.tile([128, 2 * HW], BF16, tag="xall")
            hpad = per.tile([128, PADPOS], BF16, tag="hpad")
            hT = per.tile([128, PADPOS], BF16, tag="hT")
            attnm = per.tile([128, NCHUNK * 512], BF16, tag="attnm")
            Pg = per.tile([128, HW], BF16, tag="Pg")
            attr = per.tile([128, HW], BF16, tag="attr")

            w1t0 = blobw[:, 0:CH]
            w1t1 = blobw[:, CH:2 * CH]
            w2t = blob[:, O_W2T:O_W2T + 128]
            wot = blob[:, O_WOT:O_WOT + 256]
            maskC2g = blob[:, O_M2G:O_M2G + 896]
            maskC = blob[:, O_M2G:O_M2G + 384]
            ident = blob[:, O_IDENT:O_IDENT + 128]
            onesb = blob[:, O_ONESB:O_ONESB + 128]
            b1 = blob[:, O_B1:O_B1 + 2].bitcast(F32)
            b2h = blob[:, O_B2H:O_B2H + 2].bitcast(F32)
            bout0 = blob[:, O_BOUT:O_BOUT + 2].bitcast(F32)
            bout1 = blob[:, O_BOUT + 2:O_BOUT + 4].bitcast(F32)
            d2c = blob[:, O_D2C:O_D2C + 512]

            # --- input DMAs over 4 issue queues.  Per queue: the weight /
            # blob piece that queue owns, then x blocks in consumption
            # order.  Each dma_start is packetized across all 16 HW DMA
            # engines, so few fat transfers saturate the ~250GB/s link.
            def xdma(eng, half, u):
                src = x_d[128 * half:128 * (half + 1), 1024 * u:1024 * (u + 1)]
                eng.dma_start(
                    xall[:, HW * half + 1024 * u:HW * half + 1024 * (u + 1)], src)

            nc.sync.dma_start(blobw[:, 0:CH], w1t_d[0:128, :])
            nc.scalar.dma_start(blobw[:, CH:2 * CH], w1t_d[128:256, :])
            # first blocks split fine so the first conv matmuls start early
            nc.sync.dma_start(xall[:, 0:256], x_d[0:128, 0:256])
            nc.scalar.dma_start(xall[:, HW:HW + 256], x_d[128:256, 0:256])
            # small early piece with ident/onesb/biases: nothing downstream
            # ever waits on the fat blob transfers for these
            nc.gpsimd.dma_start(blob[:, O_IDENT:O_D2C], blob_d[:, O_IDENT:O_D2C])
            nc.sync.dma_start(xall[:, 256:512], x_d[0:128, 256:512])
            nc.scalar.dma_start(xall[:, HW + 256:HW + 512], x_d[128:256, 256:512])
            nc.gpsimd.dma_start(blob[:, O_D2C:BLOBW], blob_d[:, O_D2C:BLOBW])
            nc.sync.dma_start(xall[:, 512:1024], x_d[0:128, 512:1024])
            nc.scalar.dma_start(xall[:, HW + 512:HW + 1024], x_d[128:256, 512:1024])
            xdma(nc.gpsimd, 0, 1)
            xdma(nc.sync, 1, 1)
            nc.scalar.dma_start(blob[:, 0:O_M2G], blob_d[:, 0:O_M2G])
            xdma(nc.gpsimd, 0, 2)
            xdma(nc.sync, 1, 2)
            nc.scalar.dma_start(blob[:, O_M2G:O_IDENT], blob_d[:, O_M2G:O_IDENT])
            xdma(nc.scalar, 0, 3)
            xdma(nc.gpsimd, 1, 3)

            # pad chunks (0 and 33) are identically zero
            nc.vector.memset(hpad[:, 0:128], 0.0)
            nc.vector.memset(hpad[:, PADPOS - 128:PADPOS], 0.0)
            nc.gpsimd.memset(hT[:, 0:128], 0.0)
            nc.gpsimd.memset(hT[:, PADPOS - 128:PADPOS], 0.0)



            for _rep in range(repeat):
                # ---- P1: conv1 + transposes + conv2, PE kept streaming.
                def emit_transp_group(u):
                    pt = pA.tile([128, 1024], BF16, tag="pa", name=f"pt{u}")
                    for k in range(8):
                        c = 8 * u + 1 + k
                        nc.tensor.transpose(pt[:, 128 * k:128 * (k + 1)],
                                            hpad[:, 128 * c:128 * (c + 1)],
                                            ident)
                    nc.scalar.copy(hT[:, 128 * (8 * u + 1):128 * (8 * u + 9)],
                                   pt[:])

                def emit_conv2(b):
                    pz = pB.tile([128, 512], F32, tag="pb", name=f"pz{b}")
                    nc.tensor.matmul(pz[:], w2t,
                                     hpad[:, 128 + 512 * b:128 + 512 * (b + 1)],
                                     start=True, stop=True)
                    tg = smp.tile([128, 512], BF16, tag="tg")
                    nc.scalar.activation(tg[:], pz[:], AF.Tanh, scale=0.5, bias=b2h)
                    nc.vector.tensor_scalar(
                        out=Pg[:, 512 * b:512 * (b + 1)], in0=tg[:],
                        scalar1=0.0, scalar2=1.0, op0=ALU.max, op1=ALU.add)

                def emit_conv_u(u):
                    cvt = pA.tile([128, 1024], F32, tag="pa", name=f"cv{u}")
                    for h2 in range(2):
                        # first block in 256-col pieces: starts as soon as
                        # the first fine x DMAs land
                        npc = 2 if u == 0 and h2 == 0 else 1
                        mms = []
                        for j in range(npc):
                            w = 512 // npc
                            dst = cvt[:, 512 * h2 + w * j:512 * h2 + w * (j + 1)]
                            cs = slice(1024 * u + 512 * h2 + w * j,
                                       1024 * u + 512 * h2 + w * (j + 1))
                            cs2 = slice(HW + cs.start, HW + cs.stop)
                            mms.append((dst, w1t0, xall[:, cs]))
                            mms.append((dst, w1t1, xall[:, cs2]))
                        for k, (dst, lh, rh) in enumerate(mms):
                            nc.tensor.matmul(dst, lh, rh, start=k == 0,
                                             stop=k == len(mms) - 1)
                    nc.vector.tensor_scalar(
                        out=hpad[:, 128 + 1024 * u:128 + 1024 * (u + 1)],
                        in0=cvt[:], scalar1=b1, scalar2=None, op0=ALU.add)

                deng = {}
                sampg = {}

                def ensure_group(g):
                    if g in deng or g >= NGRP:
                        return
                    deng[g] = pD.tile([128, 512], F32, tag="pd", name=f"dn{g}")
                    sampg[g] = pB.tile([128, 512], F32, tag="pb", name=f"sp{g}")
                    # den preset = D(q) via a full-width PE matmul (colsum of
                    # d2c is exactly D): also scrubs any stale pending-zero
                    # state in the bank.  samp = 0 via DVE memset (its bank
                    # was start=True full-written by a conv2 tile earlier).
                    nc.tensor.matmul(deng[g][:], onesb, d2c, start=True, stop=True)
                    nc.vector.memset(sampg[g][:], 0.0)

                def emit_score_pair(cp):
                    sc = pA.tile([128, 1024], F32, tag="pa", name=f"sc{cp}")
                    spans = []
                    for ci in range(2):
                        c = 2 * cp + ci
                        lo, hi = max(0, c - 2), min(NSUB - 1, c)
                        alo = lo - (c - 2)
                        spans.append((alo, alo + hi - lo + 1))
                        nc.tensor.matmul(
                            sc[:, 512 * ci + 128 * alo:512 * ci + 128 * (alo + hi - lo + 1)],
                            hpad[:, 128 * c:128 * (c + 1)],
                            hpad[:, 128 * (lo + 1):128 * (hi + 2)],
                            start=True, stop=True)
                    if spans == [(0, 3), (0, 3)]:
                        # one exp + one mask over both 384-wide chunk blocks,
                        # skipping the 128-col gap between them (3D APs)
                        asl = attnm[:, 1024 * cp:1024 * (cp + 1)].rearrange(
                            "p (c w) -> p c w", c=2)[:, :, 0:384]
                        sc3 = sc[:].rearrange("p (c w) -> p c w", c=2)[:, :, 0:384]
                        nc.scalar.activation(asl, sc3, AF.Exp,
                                             scale=float(SCALE))
                        nc.vector.tensor_tensor(
                            out=asl, in0=asl,
                            in1=maskC.unsqueeze(1).broadcast_to([128, 2, 384]),
                            op=ALU.mult)
                    else:
                        for ci, (a0, a1) in enumerate(spans):
                            ss = slice(512 * ci + 128 * a0, 512 * ci + 128 * a1)
                            asl = attnm[:, 1024 * cp + ss.start:1024 * cp + ss.stop]
                            nc.scalar.activation(asl, sc[:, ss], AF.Exp,
                                                 scale=float(SCALE))
                            nc.vector.tensor_tensor(
                                out=asl, in0=asl,
                                in1=maskC[:, 128 * a0:128 * a1], op=ALU.mult)

                def emit_densamp_chunk(c):
                    parts = _chunk_parts(c)
                    for g, s, e in parts:
                        aa = s - (c - 2)
                        rhs = attnm[:, 512 * c + 128 * aa:512 * c + 128 * (aa + e - s + 1)]
                        nc.tensor.matmul(
                            deng[g][:, 128 * (s - 4 * g):128 * (e + 1 - 4 * g)],
                            onesb, rhs, start=False, stop=False,
                            skip_group_check=True)
                    for g, s, e in parts:
                        aa = s - (c - 2)
                        nc.tensor.matmul(
                            sampg[g][:, 128 * (s - 4 * g):128 * (e + 1 - 4 * g)],
                            hT[:, 128 * c:128 * (c + 1)],
                            attnm[:, 512 * c + 128 * aa:512 * c + 128 * (aa + e - s + 1)],
                            start=False, stop=False, skip_group_check=True)

                def emit_finish_a(g, q0, q1):
                    # normalize subrange [128*q0, 128*q1) of group g's bank
                    gsl = slice(512 * g + 128 * q0, 512 * g + 128 * q1)
                    bsl = slice(128 * q0, 128 * q1)
                    # z = 1 / (den + D): den banks were preset with D, so a
                    # plain full-width reciprocal does it (den is broadcast
                    # across partitions by the ones stationary).  The
                    # softmax 1/2 vs gate 2x cancels via Wout/2 on host.
                    z = smp.tile([128, 512], F32, tag="z", name=f"z{g}_{q0}")
                    zc = z[:, 0:128 * (q1 - q0)]
                    nc.vector.reciprocal_approx_fast(zc, deng[g][:, bsl])
                    # Pgz = Pg * z rides on the idle gpsimd (all-SBUF); attr
                    # then needs a single DVE op reading the samp PSUM
                    pgz = smp.tile([128, 512], BF16, tag="pgz", name=f"pgz{g}_{q0}")
                    pz = pgz[:, 0:128 * (q1 - q0)]
                    nc.gpsimd.tensor_tensor(out=pz, in0=Pg[:, gsl], in1=zc,
                                            op=ALU.mult)
                    nc.vector.tensor_tensor(out=attr[:, gsl], in0=sampg[g][:, bsl],
                                            in1=pz, op=ALU.mult)

                def emit_finish_b(g, q0, q1):
                    # output conv + bias + store, one ds-pair after finish_a
                    # so the PE never waits on the DVE normalize chain
                    gsl = slice(512 * g + 128 * q0, 512 * g + 128 * q1)
                    w = 128 * (q1 - q0)
                    po = pA.tile([128, 1024], F32, tag="pa", name=f"po{g}_{q0}")
                    nc.tensor.matmul(po[:, 0:w], wot[:, 0:128], attr[:, gsl],
                                     start=True, stop=True)
                    nc.tensor.matmul(po[:, 512:512 + w], wot[:, 128:256],
                                     attr[:, gsl], start=True, stop=True)
                    ot = otp.tile([128, 1024], BF16, tag="ot")
                    nc.scalar.activation(ot[:, 0:w], po[:, 0:w],
                                         AF.Identity, bias=bout0, scale=1.0)
                    nc.scalar.activation(ot[:, 512:512 + w], po[:, 512:512 + w],
                                         AF.Identity, bias=bout1, scale=1.0)
                    for oc in range(2):
                        nc.sync.dma_start(out_d[128 * oc:128 * (oc + 1), gsl],
                                          ot[:, 512 * oc:512 * oc + w])

                fb_queue = []

                def emit_dsp(dp):
                    while fb_queue and fb_queue[0][3] < dp - 1:
                        g, q0, q1, _ = fb_queue.pop(0)
                        emit_finish_b(g, q0, q1)
                    gmax = min(NSUB - 1, 2 * dp + 3) // 4
                    for g in range(gmax + 1):
                        ensure_group(g)
                    for c in (2 * dp, 2 * dp + 1):
                        emit_densamp_chunk(c)
                        if c >= 5 and (c - 5) % 4 == 0 and c < 33:
                            g = (c - 5) // 4
                            emit_finish_a(g, 0, 4)
                            fb_queue.append((g, 0, 4, dp))
                        elif c == 31:
                            # last group in two halves to shorten the tail:
                            # subs 28,29 close at chunk 31...
                            emit_finish_a(NGRP - 1, 0, 2)
                            fb_queue.append((NGRP - 1, 0, 2, dp))
                        elif c == 33:
                            # ...subs 30,31 at chunk 33
                            emit_finish_a(NGRP - 1, 2, 4)
                            fb_queue.append((NGRP - 1, 2, 4, dp))

                # ---- interleaved schedule: conv1/transpose/conv2 (gated on
                # the x stream) with the score pipeline filling DMA-wait
                # gaps, then the chunk pipeline with den/samp trailing the
                # scores and group outputs streaming as denominators close.
                emit_conv_u(0)
                emit_conv_u(1)
                emit_transp_group(0)
                emit_conv2(0)
                emit_conv2(1)
                emit_score_pair(0)
                emit_score_pair(1)
                emit_score_pair(2)
                ensure_group(0)
                ensure_group(1)
                emit_conv_u(2)
                emit_transp_group(1)
                emit_conv2(2)
                emit_conv2(3)
                emit_score_pair(3)
                emit_score_pair(4)
                emit_dsp(0)
                emit_score_pair(5)
                emit_score_pair(6)
                emit_dsp(1)
                emit_conv_u(3)
                emit_transp_group(2)
                emit_conv2(4)
                emit_conv2(5)
                emit_score_pair(7)
                emit_dsp(2)
                emit_score_pair(8)
                emit_dsp(3)
                emit_transp_group(3)
                emit_conv2(6)
                emit_conv2(7)
                dp = 4
                for cp in range(9, 21):
                    if cp <= 16:
                        emit_score_pair(cp)
                    nds = 2 if cp == 9 else 1
                    for _ in range(nds):
                        if dp <= 16:
                            emit_dsp(dp)
                            dp += 1
                while fb_queue:
                    g, q0, q1, _ = fb_queue.pop(0)
                    emit_finish_b(g, q0, q1)

    return nc


def _prep_inputs(x, W1, b1, W2, b2, Wout, bout):
    m2g, d2c = _build_masks()
    bf = ml_dtypes.bfloat16

    blob_bf = np.zeros((128, BLOBW), dtype=bf)

    def put_bf(col, arr):
        arr = np.asarray(arr).astype(bf)
        blob_bf[:arr.shape[0], col:col + arr.shape[1]] = arr

    def put_f32(col, arr):
        arr = np.ascontiguousarray(np.asarray(arr, np.float32))
        v = arr.view(np.uint16).reshape(arr.shape[0], -1)
        blob_bf[:arr.shape[0], col:col + v.shape[1]] = v.view(bf)

    put_bf(O_W2T, np.ascontiguousarray(W2.T))
    # Wout/2 absorbs the softmax 1/2 left over from the 2*sigmoid gate
    put_bf(O_WOT, np.ascontiguousarray(np.asarray(Wout, np.float32).T * 0.5))
    put_bf(O_M2G, m2g)
    put_bf(O_IDENT, np.eye(128, dtype=np.float32))
    put_bf(O_ONESB, np.ones((128, 128), np.float32))
    put_f32(O_B1, np.asarray(b1, np.float32).reshape(CH, 1))
    put_f32(O_B2H, (0.5 * np.asarray(b2, np.float32)).reshape(CH, 1))
    put_f32(O_BOUT, np.ascontiguousarray(
        np.asarray(bout, np.float32).reshape(2, CH).T))
    put_bf(O_D2C, d2c)

    common = {
        "W1T": np.ascontiguousarray(W1.T).astype(bf),
        "blob": blob_bf,
    }
    in_maps = []
    for i in range(N):
        m = dict(common)
        m["x"] = np.ascontiguousarray(
            np.asarray(x[i], np.float32).reshape(CIN, HW)).astype(bf)
        in_maps.append(m)
    return in_maps


_CACHED = {}


def kernel(x, W1, b1, W2, b2, Wout, bout):
    if "nc" not in _CACHED:
        nc = build_nc()
        nc.finalize()
        _CACHED["nc"] = nc
    nc = _CACHED["nc"]
    in_maps = _prep_inputs(x, W1, b1, W2, b2, Wout, bout)
    res = run_bass_kernel_spmd(nc, in_maps, core_ids=list(range(N)))
    out = np.stack([np.asarray(res.results[i]["out"], dtype=np.float32)
                    .reshape(CIN, H, W) for i in range(N)])
    return out
